# revision 23
# baseline (speedup 1.0000x reference)
"""RNN(LSTM)+additive-attention language model on 8 trn2 cores.

Sharding: every core runs the full LSTM (both batches, merged into one set of
[128, 4] state tiles); core c = (b, ib) then does attention + vocab projection
for query rows [ib*128, (ib+1)*128) of batch b. Per-core row selection is done
with indirect-DMA gathers driven by per-core int32 index inputs, so all 8
cores run one identical SPMD program.

All matmuls run in bf16 (fp32 matmuls double-pump as HIGH/LOW instruction
pairs and disable fast-weight-load, which made LDWEIGHTS the kernel
bottleneck). Accumulation stays fp32 in PSUM; the softmax and LSTM cell
state stay fp32.
"""

import os
import numpy as np
import ml_dtypes
from contextlib import ExitStack

import concourse.bass as bass
import concourse.tile as tile
from concourse import bacc, mybir
from concourse.bass_utils import run_bass_kernel_spmd
from concourse.masks import make_identity

F32 = mybir.dt.float32
BF16 = mybir.dt.bfloat16
I32 = mybir.dt.int32
AF = mybir.ActivationFunctionType
AX = mybir.AxisListType
BFNP = ml_dtypes.bfloat16

B, T, E, H, VOCAB = 2, 512, 256, 256, 32000
NCORES = 8
QB = 128          # query rows per core
VB = 500          # vocab cols per projection block
NVB = VOCAB // VB  # 64


def build():
    nc = bacc.Bacc("TRN2", num_devices=NCORES)

    emb_e = nc.declare_dram_parameter("emb", [VOCAB, E], F32, isOutput=False)
    xt_e = nc.declare_dram_parameter("xt", [128, 8], I32, isOutput=False)
    wih_e = nc.declare_dram_parameter("wihT", [E, 4 * H], BF16, isOutput=False)
    whh_e = nc.declare_dram_parameter("whhT", [H, 4 * H], BF16, isOutput=False)
    bT_e = nc.declare_dram_parameter("biasT", [128, 8], F32, isOutput=False)
    w1_e = nc.declare_dram_parameter("w1T", [H, H], BF16, isOutput=False)
    w2_e = nc.declare_dram_parameter("w2T", [H, H], BF16, isOutput=False)
    b12_e = nc.declare_dram_parameter("b12", [1, H], BF16, isOutput=False)
    vt_e = nc.declare_dram_parameter("vt", [128, 2], BF16, isOutput=False)
    wfc_e = nc.declare_dram_parameter("wfcT", [2 * H, VOCAB], BF16, isOutput=False)
    qi_e = nc.declare_dram_parameter("qi", [128, 1], I32, isOutput=False)
    ki_e = nc.declare_dram_parameter("ki", [128, 4], I32, isOutput=False)
    mask_e = nc.declare_dram_parameter("mask", [128, T], F32, isOutput=False)
    out_e = nc.declare_dram_parameter("out", [QB, VOCAB], F32, isOutput=True)

    a_dram = nc.dram_tensor("a_scr", [B * T, H], F32)
    b_dram = nc.dram_tensor("b_scr", [B * T, H], F32)
    o_dram = nc.dram_tensor("o_scr", [B * T, H], BF16)

    with tile.TileContext(nc) as tc, ExitStack() as ctx:
        cp = ctx.enter_context(tc.tile_pool(name="cp", bufs=1))
        sp = ctx.enter_context(tc.tile_pool(name="sp", bufs=3))
        wp = ctx.enter_context(tc.tile_pool(name="wp", bufs=8))
        pp = ctx.enter_context(tc.tile_pool(name="pp", bufs=2, space="PSUM"))

        # ---- constants / params ----
        ident = cp.tile([128, 128], F32)
        make_identity(nc, ident)
        identb = cp.tile([128, 128], BF16)
        nc.vector.tensor_copy(out=identb, in_=ident)
        ones_s = cp.tile([1, 128], BF16)
        nc.vector.memset(ones_s, 1.0)

        wih_s = cp.tile([128, 2 * 4 * H], BF16)   # col = kc*1024 + g
        whh_s = cp.tile([128, 2 * 4 * H], BF16)
        for kc in range(2):
            nc.sync.dma_start(out=wih_s[:, kc * 1024:(kc + 1) * 1024],
                              in_=wih_e[kc * 128:(kc + 1) * 128, :])
            nc.sync.dma_start(out=whh_s[:, kc * 1024:(kc + 1) * 1024],
                              in_=whh_e[kc * 128:(kc + 1) * 128, :])
        biasT_s = cp.tile([128, 8], F32)
        nc.sync.dma_start(out=biasT_s, in_=bT_e[:])
        w1_s = cp.tile([128, 2 * H], BF16)        # col = hcin*256 + hout
        w2_s = cp.tile([128, 2 * H], BF16)
        for kc in range(2):
            nc.sync.dma_start(out=w1_s[:, kc * H:(kc + 1) * H],
                              in_=w1_e[kc * 128:(kc + 1) * 128, :])
            nc.sync.dma_start(out=w2_s[:, kc * H:(kc + 1) * H],
                              in_=w2_e[kc * 128:(kc + 1) * 128, :])
        b12_s = cp.tile([1, H], BF16)
        nc.sync.dma_start(out=b12_s, in_=b12_e[:])
        vt_s = cp.tile([128, 2], BF16)
        nc.sync.dma_start(out=vt_s, in_=vt_e[:])
        xt_s = cp.tile([128, 8], I32)
        nc.sync.dma_start(out=xt_s, in_=xt_e[:])
        qi_s = cp.tile([128, 1], I32)
        nc.sync.dma_start(out=qi_s, in_=qi_e[:])
        ki_s = cp.tile([128, 4], I32)
        nc.sync.dma_start(out=ki_s, in_=ki_e[:])
        mask_s = cp.tile([128, T], F32)
        nc.sync.dma_start(out=mask_s, in_=mask_e[:])

        # ---- embedding gather + transpose -> xeT[b] [128, 2ec*512] bf16 ----
        xeT = [cp.tile([128, 2 * T], BF16, name=f"xeT{b}") for b in range(B)]
        for tch in range(4):
            for b in range(B):
                xe_rows = sp.tile([128, E], F32)
                nc.gpsimd.indirect_dma_start(
                    out=xe_rows, out_offset=None, in_=emb_e[:],
                    in_offset=bass.IndirectOffsetOnAxis(
                        ap=xt_s[:, b * 4 + tch:b * 4 + tch + 1], axis=0))
                for ec in range(2):
                    trp = pp.tile([128, T], F32, name="big", bufs=2)[:, 0:128]
                    nc.tensor.transpose(trp, xe_rows[:, ec * 128:(ec + 1) * 128], ident)
                    nc.scalar.activation(
                        xeT[b][:, ec * T + tch * 128: ec * T + (tch + 1) * 128],
                        trp, AF.Copy)

        # ---- gx precompute: gxT [128, T*16] bf16, col = t*16 + gc*2 + b ----
        # chunked over t so the LSTM can start after the first chunk
        gxT = cp.tile([128, T * 16], BF16)
        TCH = 64
        for tch in range(T // TCH):
            for b in range(B):
                for gc in range(8):
                    gx_ps = pp.tile([128, T], F32, name="big", bufs=2)[:, 0:TCH]
                    for ec in range(2):
                        nc.tensor.matmul(
                            gx_ps,
                            wih_s[:, ec * 1024 + gc * 128: ec * 1024 + (gc + 1) * 128],
                            xeT[b][:, ec * T + tch * TCH: ec * T + (tch + 1) * TCH],
                            start=(ec == 0), stop=(ec == 1))
                    off = tch * TCH * 16 + gc * 2 + b
                    nc.vector.tensor_scalar(
                        out=gxT[:, off: off + 16 * (TCH - 1) + 1: 16], in0=gx_ps,
                        scalar1=biasT_s[:, gc:gc + 1], scalar2=None,
                        op0=mybir.AluOpType.add)

        # ---- LSTM ----
        # outT_all col = t*4 + kc*2 + b (h in bf16, written directly by the
        # h-mul). act tiles (ping-pong): cols 0:4=i, 4:8=f, 8:12=o,
        # 12:16=tanh(g), 16:20=c_prev (f32). gx is injected into PSUM with an
        # identity matmul so the gate activations read PSUM directly.
        # g-gate matmuls run first so tanh(g) overlaps the remaining matmuls.
        outT_all = cp.tile([128, 4 * T], BF16)
        acts = [cp.tile([128, 20], F32, name=f"act{i}") for i in range(2)]
        nc.vector.memset(acts[0][:, 16:20], 0.0)
        z4 = cp.tile([128, 4], BF16)
        nc.vector.memset(z4, 0.0)
        # gate order in tiles (host perm): i, f, o, g -> gc blocks
        #   i: gc0,1  f: gc2,3  o: gc4,5  g: gc6,7
        # gates go to three separate PSUM tiles (banks) so each activation
        # only waits for its own gate matmuls (PSUM deps are bank-level):
        # g first (tanh overlaps i/f matmuls), then i,f, then o.
        MM_ORDER = [6, 7, 0, 1, 2, 3, 4, 5]

        def outv(b, hc, t0, n):
            """[128, n] bf16 view of outputs: h-chunk hc, batch b, t0..t0+n."""
            s = t0 * 4 + hc * 2 + b
            return outT_all[:, s: s + 4 * (n - 1) + 1: 4]

        def features(tch):
            """a/b features + output rows -> DRAM for t-chunk tch.

            Emitted right after the LSTM steps that produce chunk tch, so the
            scheduler can backfill them into LSTM idle slots.
            """
            for b in range(B):
                for w_s, dram, with_bias in (
                        (w1_s, a_dram, True), (w2_s, b_dram, False)):
                    f_t = pp.tile([128, T], F32, name="big", bufs=2)
                    f_ps = f_t[:, 0:H]
                    for hc in range(2):
                        nc.tensor.matmul(
                            f_ps,
                            outv(b, hc, tch * 128, 128),
                            w_s[:, hc * H:(hc + 1) * H],
                            start=(hc == 0),
                            stop=(False if with_bias else hc == 1))
                    if with_bias:
                        nc.tensor.matmul(f_ps, ones_s, b12_s, start=False, stop=True)
                    f_sb = sp.tile([128, H], F32, name="f_sb", bufs=4)
                    nc.vector.tensor_copy(out=f_sb, in_=f_ps)
                    nc.sync.dma_start(
                        out=dram[b * T + tch * 128: b * T + (tch + 1) * 128, :],
                        in_=f_sb)
                o_sb = sp.tile([128, H], BF16, name="o_sb", bufs=4)
                for hc in range(2):
                    trp = pp.tile([128, T], BF16, name="bigb", bufs=1)[:, 0:128]
                    nc.tensor.transpose(
                        trp, outv(b, hc, tch * 128, 128), identb)
                    nc.scalar.activation(o_sb[:, hc * 128:(hc + 1) * 128], trp, AF.Copy)
                nc.sync.dma_start(
                    out=o_dram[b * T + tch * 128: b * T + (tch + 1) * 128, :],
                    in_=o_sb)

        for t in range(T):
            A = acts[t % 2]
            An = acts[(t + 1) % 2]
            hT = z4 if t == 0 else outT_all[:, (t - 1) * 4: t * 4]
            g_if = pp.tile([128, 8], F32, name="gps_if", bufs=1)
            g_o = pp.tile([128, 4], F32, name="gps_o", bufs=1)
            g_g = pp.tile([128, 4], F32, name="gps_g", bufs=1)

            def gview(gc):
                if gc < 4:
                    return g_if[:, gc * 2: gc * 2 + 2]
                if gc < 6:
                    return g_o[:, (gc - 4) * 2: (gc - 4) * 2 + 2]
                return g_g[:, (gc - 6) * 2: (gc - 6) * 2 + 2]

            nc.tensor.matmul(g_g, identb, gxT[:, t * 16 + 12: t * 16 + 16],
                             start=True, stop=False, skip_group_check=True)
            nc.tensor.matmul(g_if, identb, gxT[:, t * 16: t * 16 + 8],
                             start=True, stop=False, skip_group_check=True)
            nc.tensor.matmul(g_o, identb, gxT[:, t * 16 + 8: t * 16 + 12],
                             start=True, stop=False, skip_group_check=True)
            for kc in range(2):
                for i, gc in enumerate(MM_ORDER):
                    nc.tensor.matmul(
                        gview(gc),
                        whh_s[:, kc * 1024 + gc * 128: kc * 1024 + (gc + 1) * 128],
                        hT[:, kc * 2: kc * 2 + 2],
                        start=False, stop=(kc == 1 and i == 7),
                        skip_group_check=True)
            nc.scalar.activation(A[:, 12:16], g_g, AF.Tanh)
            nc.scalar.activation(A[:, 0:8], g_if, AF.Sigmoid)
            nc.scalar.activation(A[:, 8:12], g_o, AF.Sigmoid)
            prod = sp.tile([128, 8], F32, name="prod")
            nc.vector.tensor_mul(out=prod, in0=A[:, 0:8], in1=A[:, 12:20])
            nc.vector.tensor_add(out=An[:, 16:20], in0=prod[:, 0:4],
                                 in1=prod[:, 4:8])
            thc = sp.tile([128, 4], F32, name="thc")
            nc.scalar.activation(thc, An[:, 16:20], AF.Tanh)
            # h write split by k-chunk so the next step's kc=0 matmuls can
            # start while the kc=1 half is still being written
            nc.vector.tensor_mul(out=outT_all[:, t * 4: t * 4 + 2],
                                 in0=A[:, 8:10], in1=thc[:, 0:2])
            nc.vector.tensor_mul(out=outT_all[:, t * 4 + 2: t * 4 + 4],
                                 in0=A[:, 10:12], in1=thc[:, 2:4])
            if (t + 1) % 128 == 0:
                features((t + 1) // 128 - 1)

        # ---- gathers for this core's (b, iblock) ----
        aq_rows = cp.tile([128, H], F32)
        nc.gpsimd.indirect_dma_start(
            out=aq_rows, out_offset=None, in_=a_dram[:],
            in_offset=bass.IndirectOffsetOnAxis(ap=qi_s[:, 0:1], axis=0))
        oq_rows = cp.tile([128, H], BF16)
        nc.gpsimd.indirect_dma_start(
            out=oq_rows, out_offset=None, in_=o_dram[:],
            in_offset=bass.IndirectOffsetOnAxis(ap=qi_s[:, 0:1], axis=0))
        aq_s = cp.tile([128, H], BF16)    # col = hc*128 + q
        oqT_s = cp.tile([128, H], BF16)
        for hc in range(2):
            trp = pp.tile([128, T], F32, name="big", bufs=2)[:, 0:128]
            nc.tensor.transpose(trp, aq_rows[:, hc * 128:(hc + 1) * 128], ident)
            nc.scalar.activation(aq_s[:, hc * 128:(hc + 1) * 128], trp, AF.Copy)
            trp2 = pp.tile([128, T], BF16, name="bigb", bufs=1)[:, 0:128]
            nc.tensor.transpose(trp2, oq_rows[:, hc * 128:(hc + 1) * 128], identb)
            nc.scalar.activation(oqT_s[:, hc * 128:(hc + 1) * 128], trp2, AF.Copy)

        bT_s = cp.tile([128, 2 * T], BF16)  # col = hc*512 + j
        our = [cp.tile([128, H], BF16, name=f"our{jc}") for jc in range(4)]
        for jc in range(4):
            b_rows = sp.tile([128, H], F32, name="b_rows", bufs=4)
            nc.gpsimd.indirect_dma_start(
                out=b_rows, out_offset=None, in_=b_dram[:],
                in_offset=bass.IndirectOffsetOnAxis(ap=ki_s[:, jc:jc + 1], axis=0))
            nc.gpsimd.indirect_dma_start(
                out=our[jc], out_offset=None, in_=o_dram[:],
                in_offset=bass.IndirectOffsetOnAxis(ap=ki_s[:, jc:jc + 1], axis=0))
            for hc in range(2):
                trp = pp.tile([128, T], F32, name="big", bufs=2)[:, 0:128]
                nc.tensor.transpose(trp, b_rows[:, hc * 128:(hc + 1) * 128], ident)
                nc.scalar.activation(
                    bT_s[:, hc * T + jc * 128: hc * T + (jc + 1) * 128], trp, AF.Copy)

        # ---- scores + softmax ----
        sm_s = cp.tile([128, T], F32)
        for q in range(QB):
            sc1 = pp.tile([1, T], F32, name="sc1", bufs=2)
            for hc in range(2):
                th = sp.tile([128, T], BF16, name="th", bufs=4)
                nc.scalar.activation(
                    th, bT_s[:, hc * T:(hc + 1) * T], AF.Tanh,
                    bias=aq_s[:, hc * 128 + q: hc * 128 + q + 1])
                nc.tensor.matmul(sc1, vt_s[:, hc:hc + 1], th,
                                 start=(hc == 0), stop=(hc == 1))
            scq = sp.tile([1, T], F32, name="scq", bufs=4)
            nc.vector.tensor_copy(out=scq, in_=sc1)
            nc.sync.dma_start(out=sm_s[q:q + 1, :], in_=scq)

        # ---- projection, oq half: emitted after the scores loop so it
        # backfills PE idle slots during the (ACT-bound) scores phase;
        # partial logits staged in SBUF as bf16 ----
        partial = cp.tile([128, NVB * VB], BF16)
        for vb in range(NVB):
            wt1 = wp.tile([128, 2 * VB], BF16, name="wt1")
            nc.sync.dma_start(
                out=wt1[:].rearrange("p (a v) -> p a v", a=2),
                in_=wfc_e[0:256, vb * VB:(vb + 1) * VB].rearrange(
                    "(a p) v -> p a v", p=128))
            ps = pp.tile([128, T], F32, name="big", bufs=2)[:, 0:VB]
            for kc in range(2):
                nc.tensor.matmul(ps, oqT_s[:, kc * 128:(kc + 1) * 128],
                                 wt1[:, kc * VB:(kc + 1) * VB],
                                 start=(kc == 0), stop=(kc == 1))
            nc.vector.tensor_copy(out=partial[:, vb * VB:(vb + 1) * VB], in_=ps)

        nc.vector.tensor_add(out=sm_s, in0=sm_s, in1=mask_s)
        nmx = cp.tile([128, 1], F32)
        nc.vector.reduce_max(nmx, sm_s, axis=AX.X, negate=True)
        ex_s = cp.tile([128, T], F32)
        ssum = cp.tile([128, 1], F32)
        nc.scalar.activation(ex_s, sm_s, AF.Exp, bias=nmx, accum_out=ssum)
        rs = cp.tile([128, 1], F32)
        nc.vector.reciprocal(rs, ssum)
        at_s = cp.tile([128, T], F32)
        nc.vector.tensor_scalar(out=at_s, in0=ex_s, scalar1=rs, scalar2=None,
                                op0=mybir.AluOpType.mult)

        # ---- context: ctxT [h, q] ----
        ctx_ps = pp.tile([128, T], F32, name="big", bufs=2)[:, 0:H]
        atT = [cp.tile([128, 128], BF16, name=f"atT{jc}") for jc in range(4)]
        for jc in range(4):
            trp = pp.tile([128, T], F32, name="big", bufs=2)[:, 0:128]
            nc.tensor.transpose(trp, at_s[:, jc * 128:(jc + 1) * 128], ident)
            nc.scalar.activation(atT[jc], trp, AF.Copy)
        for hc in range(2):
            for jc in range(4):
                nc.tensor.matmul(ctx_ps[:, hc * 128:(hc + 1) * 128],
                                 our[jc][:, hc * 128:(hc + 1) * 128], atT[jc],
                                 start=(jc == 0), stop=(jc == 3))
        ctxT_s = cp.tile([128, H], BF16)
        nc.vector.tensor_copy(out=ctxT_s, in_=ctx_ps)

        # ---- projection, ctx half + staged oq partial ----
        for vb in range(NVB):
            wt2 = wp.tile([128, 2 * VB], BF16, name="wt2")
            nc.sync.dma_start(
                out=wt2[:].rearrange("p (a v) -> p a v", a=2),
                in_=wfc_e[256:512, vb * VB:(vb + 1) * VB].rearrange(
                    "(a p) v -> p a v", p=128))
            # alternate between two PSUM tags (4 banks total) for a deeper
            # matmul/add/store pipeline
            lg_ps = pp.tile([128, T], F32, name=("big" if vb % 2 else "sc1"),
                            bufs=2)[:, 0:VB]
            for kc in range(2):
                nc.tensor.matmul(lg_ps, ctxT_s[:, kc * 128:(kc + 1) * 128],
                                 wt2[:, kc * VB:(kc + 1) * VB],
                                 start=(kc == 0), stop=(kc == 1))
            lg_sb = sp.tile([128, VB], F32, name="lg_sb", bufs=4)
            nc.vector.tensor_add(out=lg_sb, in0=lg_ps,
                                 in1=partial[:, vb * VB:(vb + 1) * VB])
            nc.sync.dma_start(out=out_e[:, vb * VB:(vb + 1) * VB], in_=lg_sb)

    nc.finalize()
    return nc


_NC = None


def _get_nc():
    global _NC
    if _NC is None:
        _NC = build()
    return _NC


def _prep(inputs):
    x = np.asarray(inputs["x"])
    perm = np.concatenate([np.arange(0, 512), np.arange(768, 1024),
                           np.arange(512, 768)])
    wihT = np.ascontiguousarray(np.asarray(inputs["W_ih"])[perm].T.astype(BFNP))
    whhT = np.ascontiguousarray(np.asarray(inputs["W_hh"])[perm].T.astype(BFNP))
    bias = (np.asarray(inputs["b_ih"]) + np.asarray(inputs["b_hh"]))[perm]
    biasT = np.ascontiguousarray(bias.reshape(8, 128).T)
    w1T = np.ascontiguousarray(np.asarray(inputs["W1"]).T.astype(BFNP))
    w2T = np.ascontiguousarray(np.asarray(inputs["W2"]).T.astype(BFNP))
    b12 = (np.asarray(inputs["b1"]) + np.asarray(inputs["b2"])).reshape(1, H)
    vt = np.ascontiguousarray(np.asarray(inputs["V"])[0].reshape(2, 128).T.astype(BFNP))
    wfcT = np.ascontiguousarray(np.asarray(inputs["Wfc"]).T.astype(BFNP))
    xt = np.zeros((128, 8), np.int32)
    for b in range(B):
        for tch in range(4):
            xt[:, b * 4 + tch] = x[b, tch * 128:(tch + 1) * 128]
    common = dict(
        emb=np.ascontiguousarray(np.asarray(inputs["emb"], np.float32)),
        xt=xt, wihT=wihT, whhT=whhT,
        biasT=np.ascontiguousarray(biasT.astype(np.float32)),
        w1T=w1T, w2T=w2T,
        b12=np.ascontiguousarray(b12.astype(BFNP)), vt=vt,
        wfcT=wfcT)
    r = np.arange(128)
    in_maps = []
    for c in range(NCORES):
        b, ib = divmod(c, 4)
        qi = (b * T + ib * 128 + r).astype(np.int32).reshape(128, 1)
        ki = np.stack([(b * T + jc * 128 + r).astype(np.int32)
                       for jc in range(4)], axis=1)
        mask = np.where(np.arange(T)[None, :] <= (ib * 128 + r)[:, None],
                        np.float32(0.0), np.float32(-1e30)).astype(np.float32)
        m = dict(common)
        m.update(qi=qi, ki=np.ascontiguousarray(ki), mask=mask)
        in_maps.append(m)
    return in_maps


LAST = None


def assemble(results, inputs):
    bfc = np.asarray(inputs["bfc"], np.float32)
    logits = np.empty((B, T, VOCAB), np.float32)
    for c in range(NCORES):
        b, ib = divmod(c, 4)
        logits[b, ib * 128:(ib + 1) * 128, :] = results[c]["out"]
    logits += bfc[None, None, :]
    return logits


def kernel(**inputs):
    global LAST
    nc = _get_nc()
    in_maps = _prep(inputs)
    trace = bool(os.environ.get("KERNEL_TRACE"))
    try:
        br = run_bass_kernel_spmd(nc, in_maps, list(range(NCORES)), trace=trace)
    except Exception:
        if not trace:
            raise
        br = run_bass_kernel_spmd(nc, in_maps, list(range(NCORES)), trace=False)
    LAST = br
    return assemble(br.results, inputs)


if __name__ == "__main__":
    build()
    print("build ok")


# revision 25
# speedup vs baseline: 1.2162x; 1.2162x over previous
"""RNN(LSTM)+additive-attention language model on 8 trn2 cores.

Sharding: every core runs the full LSTM (both batches, merged into one set of
[128, 4] state tiles); core c = (b, ib) then does attention + vocab projection
for query rows [ib*128, (ib+1)*128) of batch b. Per-core row selection is done
with indirect-DMA gathers driven by per-core int32 index inputs, so all 8
cores run one identical SPMD program.

All matmuls run in bf16 (fp32 matmuls double-pump as HIGH/LOW instruction
pairs and disable fast-weight-load, which made LDWEIGHTS the kernel
bottleneck). Accumulation stays fp32 in PSUM; the softmax and LSTM cell
state stay fp32.
"""

import os
import numpy as np
import ml_dtypes
from contextlib import ExitStack

import concourse.bass as bass
import concourse.tile as tile
from concourse import bacc, mybir
from concourse.bass_utils import run_bass_kernel_spmd
from concourse.masks import make_identity

F32 = mybir.dt.float32
BF16 = mybir.dt.bfloat16
I32 = mybir.dt.int32
AF = mybir.ActivationFunctionType
AX = mybir.AxisListType
BFNP = ml_dtypes.bfloat16

B, T, E, H, VOCAB = 2, 512, 256, 256, 32000
NCORES = 8
QB = 128          # query rows per core
VB = 500          # vocab cols per projection block
NVB = VOCAB // VB  # 64


def build():
    nc = bacc.Bacc("TRN2", num_devices=NCORES)

    emb_e = nc.declare_dram_parameter("emb", [VOCAB, E], F32, isOutput=False)
    xt_e = nc.declare_dram_parameter("xt", [128, 8], I32, isOutput=False)
    wih_e = nc.declare_dram_parameter("wihT", [E, 4 * H], BF16, isOutput=False)
    whh_e = nc.declare_dram_parameter("whhT", [H, 4 * H], BF16, isOutput=False)
    bT_e = nc.declare_dram_parameter("biasT", [128, 8], F32, isOutput=False)
    w1_e = nc.declare_dram_parameter("w1T", [H, H], BF16, isOutput=False)
    w2_e = nc.declare_dram_parameter("w2T", [H, H], BF16, isOutput=False)
    b12_e = nc.declare_dram_parameter("b12", [1, H], BF16, isOutput=False)
    vt_e = nc.declare_dram_parameter("vt", [128, 2], BF16, isOutput=False)
    wfc_e = nc.declare_dram_parameter("wfcT", [2 * H, VOCAB], BF16, isOutput=False)
    qi_e = nc.declare_dram_parameter("qi", [128, 1], I32, isOutput=False)
    ki_e = nc.declare_dram_parameter("ki", [128, 4], I32, isOutput=False)
    mask_e = nc.declare_dram_parameter("mask", [128, T], F32, isOutput=False)
    out_e = nc.declare_dram_parameter("out", [QB, VOCAB], F32, isOutput=True)

    a_dram = nc.dram_tensor("a_scr", [B * T, H], F32)
    b_dram = nc.dram_tensor("b_scr", [B * T, H], F32)
    o_dram = nc.dram_tensor("o_scr", [B * T, H], BF16)

    with tile.TileContext(nc) as tc, ExitStack() as ctx:
        cp = ctx.enter_context(tc.tile_pool(name="cp", bufs=1))
        sp = ctx.enter_context(tc.tile_pool(name="sp", bufs=3))
        wp = ctx.enter_context(tc.tile_pool(name="wp", bufs=8))
        pp = ctx.enter_context(tc.tile_pool(name="pp", bufs=2, space="PSUM"))

        # ---- constants / params ----
        ident = cp.tile([128, 128], F32)
        make_identity(nc, ident)
        identb = cp.tile([128, 128], BF16)
        nc.vector.tensor_copy(out=identb, in_=ident)
        ones_s = cp.tile([1, 128], BF16)
        nc.vector.memset(ones_s, 1.0)

        wih_s = cp.tile([128, 2 * 4 * H], BF16)   # col = kc*1024 + g
        whh_s = cp.tile([128, 2 * 4 * H], BF16)
        for kc in range(2):
            nc.sync.dma_start(out=wih_s[:, kc * 1024:(kc + 1) * 1024],
                              in_=wih_e[kc * 128:(kc + 1) * 128, :])
            nc.sync.dma_start(out=whh_s[:, kc * 1024:(kc + 1) * 1024],
                              in_=whh_e[kc * 128:(kc + 1) * 128, :])
        biasT_s = cp.tile([128, 8], F32)
        nc.sync.dma_start(out=biasT_s, in_=bT_e[:])
        w1_s = cp.tile([128, 2 * H], BF16)        # col = hcin*256 + hout
        w2_s = cp.tile([128, 2 * H], BF16)
        for kc in range(2):
            nc.sync.dma_start(out=w1_s[:, kc * H:(kc + 1) * H],
                              in_=w1_e[kc * 128:(kc + 1) * 128, :])
            nc.sync.dma_start(out=w2_s[:, kc * H:(kc + 1) * H],
                              in_=w2_e[kc * 128:(kc + 1) * 128, :])
        b12_s = cp.tile([1, H], BF16)
        nc.sync.dma_start(out=b12_s, in_=b12_e[:])
        vt_s = cp.tile([128, 2], BF16)
        nc.sync.dma_start(out=vt_s, in_=vt_e[:])
        xt_s = cp.tile([128, 8], I32)
        nc.sync.dma_start(out=xt_s, in_=xt_e[:])
        qi_s = cp.tile([128, 1], I32)
        nc.sync.dma_start(out=qi_s, in_=qi_e[:])
        ki_s = cp.tile([128, 4], I32)
        nc.sync.dma_start(out=ki_s, in_=ki_e[:])
        mask_s = cp.tile([128, T], F32)
        nc.sync.dma_start(out=mask_s, in_=mask_e[:])

        # ---- embedding gather + transpose -> xeT[b] [128, 2ec*512] bf16 ----
        xeT = [cp.tile([128, 2 * T], BF16, name=f"xeT{b}") for b in range(B)]
        for tch in range(4):
            for b in range(B):
                xe_rows = sp.tile([128, E], F32)
                nc.gpsimd.indirect_dma_start(
                    out=xe_rows, out_offset=None, in_=emb_e[:],
                    in_offset=bass.IndirectOffsetOnAxis(
                        ap=xt_s[:, b * 4 + tch:b * 4 + tch + 1], axis=0))
                for ec in range(2):
                    trp = pp.tile([128, T], F32, name="big", bufs=2)[:, 0:128]
                    nc.tensor.transpose(trp, xe_rows[:, ec * 128:(ec + 1) * 128], ident)
                    nc.scalar.activation(
                        xeT[b][:, ec * T + tch * 128: ec * T + (tch + 1) * 128],
                        trp, AF.Copy)

        # ---- gx precompute: gxT [128, T*16] bf16, col = t*16 + gc*2 + b ----
        # chunked over t so the LSTM can start after the first chunk
        gxT = cp.tile([128, T * 16], BF16)
        TCH = 64
        for tch in range(T // TCH):
            for b in range(B):
                for gc in range(8):
                    gx_ps = pp.tile([128, T], F32, name="big", bufs=2)[:, 0:TCH]
                    for ec in range(2):
                        nc.tensor.matmul(
                            gx_ps,
                            wih_s[:, ec * 1024 + gc * 128: ec * 1024 + (gc + 1) * 128],
                            xeT[b][:, ec * T + tch * TCH: ec * T + (tch + 1) * TCH],
                            start=(ec == 0), stop=(ec == 1))
                    off = tch * TCH * 16 + gc * 2 + b
                    nc.vector.tensor_scalar(
                        out=gxT[:, off: off + 16 * (TCH - 1) + 1: 16], in0=gx_ps,
                        scalar1=biasT_s[:, gc:gc + 1], scalar2=None,
                        op0=mybir.AluOpType.add)

        # ---- LSTM ----
        # outT_all col = t*4 + kc*2 + b (h in bf16, written directly by the
        # h-mul). act tiles (ping-pong): cols 0:4=i, 4:8=f, 8:12=o,
        # 12:16=tanh(g), 16:20=c_prev (f32). gx is injected into PSUM with an
        # identity matmul so the gate activations read PSUM directly.
        # g-gate matmuls run first so tanh(g) overlaps the remaining matmuls.
        outT_all = cp.tile([128, 4 * T], BF16)
        acts = [cp.tile([128, 20], F32, name=f"act{i}") for i in range(2)]
        nc.vector.memset(acts[0][:, 16:20], 0.0)
        z4 = cp.tile([128, 4], BF16)
        nc.vector.memset(z4, 0.0)
        # gate order in tiles (host perm): i, f, o, g -> gc blocks
        #   i: gc0,1  f: gc2,3  o: gc4,5  g: gc6,7
        # gates go to three separate PSUM tiles (banks) so each activation
        # only waits for its own gate matmuls (PSUM deps are bank-level):
        # g first (tanh overlaps i/f matmuls), then i,f, then o.
        MM_ORDER = [6, 7, 0, 1, 2, 3, 4, 5]

        def outv(b, hc, t0, n):
            """[128, n] bf16 view of outputs: h-chunk hc, batch b, t0..t0+n."""
            s = t0 * 4 + hc * 2 + b
            return outT_all[:, s: s + 4 * (n - 1) + 1: 4]

        def features(tch):
            """a/b features + output rows -> DRAM for t-chunk tch.

            Emitted right after the LSTM steps that produce chunk tch, so the
            scheduler can backfill them into LSTM idle slots.
            """
            for b in range(B):
                for w_s, dram, with_bias in (
                        (w1_s, a_dram, True), (w2_s, b_dram, False)):
                    f_t = pp.tile([128, T], F32, name="big", bufs=2)
                    f_ps = f_t[:, 0:H]
                    for hc in range(2):
                        nc.tensor.matmul(
                            f_ps,
                            outv(b, hc, tch * 128, 128),
                            w_s[:, hc * H:(hc + 1) * H],
                            start=(hc == 0),
                            stop=(False if with_bias else hc == 1))
                    if with_bias:
                        nc.tensor.matmul(f_ps, ones_s, b12_s, start=False, stop=True)
                    f_sb = sp.tile([128, H], F32, name="f_sb", bufs=4)
                    nc.vector.tensor_copy(out=f_sb, in_=f_ps)
                    nc.sync.dma_start(
                        out=dram[b * T + tch * 128: b * T + (tch + 1) * 128, :],
                        in_=f_sb)
                o_sb = sp.tile([128, H], BF16, name="o_sb", bufs=4)
                for hc in range(2):
                    trp = pp.tile([128, T], BF16, name="bigb", bufs=1)[:, 0:128]
                    nc.tensor.transpose(
                        trp, outv(b, hc, tch * 128, 128), identb)
                    nc.scalar.activation(o_sb[:, hc * 128:(hc + 1) * 128], trp, AF.Copy)
                nc.sync.dma_start(
                    out=o_dram[b * T + tch * 128: b * T + (tch + 1) * 128, :],
                    in_=o_sb)

        for t in range(T):
            A = acts[t % 2]
            An = acts[(t + 1) % 2]
            hT = z4 if t == 0 else outT_all[:, (t - 1) * 4: t * 4]
            g_if = pp.tile([128, 8], F32, name="gps_if", bufs=1)
            g_o = pp.tile([128, 4], F32, name="gps_o", bufs=1)
            g_g = pp.tile([128, 4], F32, name="gps_g", bufs=1)

            def gview(gc):
                if gc < 4:
                    return g_if[:, gc * 2: gc * 2 + 2]
                if gc < 6:
                    return g_o[:, (gc - 4) * 2: (gc - 4) * 2 + 2]
                return g_g[:, (gc - 6) * 2: (gc - 6) * 2 + 2]

            nc.tensor.matmul(g_g, identb, gxT[:, t * 16 + 12: t * 16 + 16],
                             start=True, stop=False, skip_group_check=True)
            nc.tensor.matmul(g_if, identb, gxT[:, t * 16: t * 16 + 8],
                             start=True, stop=False, skip_group_check=True)
            nc.tensor.matmul(g_o, identb, gxT[:, t * 16 + 8: t * 16 + 12],
                             start=True, stop=False, skip_group_check=True)
            for i, gc in enumerate(MM_ORDER):
                for kc in range(2):
                    nc.tensor.matmul(
                        gview(gc),
                        whh_s[:, kc * 1024 + gc * 128: kc * 1024 + (gc + 1) * 128],
                        hT[:, kc * 2: kc * 2 + 2],
                        start=False, stop=(i == 7 and kc == 1),
                        skip_group_check=True)
            nc.scalar.activation(A[:, 12:16], g_g, AF.Tanh)
            nc.scalar.activation(A[:, 0:8], g_if, AF.Sigmoid)
            nc.scalar.activation(A[:, 8:12], g_o, AF.Sigmoid)
            prod = sp.tile([128, 8], F32, name="prod")
            nc.vector.tensor_mul(out=prod, in0=A[:, 0:8], in1=A[:, 12:20])
            nc.vector.tensor_add(out=An[:, 16:20], in0=prod[:, 0:4],
                                 in1=prod[:, 4:8])
            thc = sp.tile([128, 4], F32, name="thc")
            nc.scalar.activation(thc, An[:, 16:20], AF.Tanh)
            nc.vector.tensor_mul(out=outT_all[:, t * 4:(t + 1) * 4],
                                 in0=A[:, 8:12], in1=thc)
            if (t + 1) % 128 == 0:
                features((t + 1) // 128 - 1)

        # ---- gathers for this core's (b, iblock) ----
        aq_rows = cp.tile([128, H], F32)
        nc.gpsimd.indirect_dma_start(
            out=aq_rows, out_offset=None, in_=a_dram[:],
            in_offset=bass.IndirectOffsetOnAxis(ap=qi_s[:, 0:1], axis=0))
        oq_rows = cp.tile([128, H], BF16)
        nc.gpsimd.indirect_dma_start(
            out=oq_rows, out_offset=None, in_=o_dram[:],
            in_offset=bass.IndirectOffsetOnAxis(ap=qi_s[:, 0:1], axis=0))
        aq_s = cp.tile([128, H], BF16)    # col = hc*128 + q
        oqT_s = cp.tile([128, H], BF16)
        for hc in range(2):
            trp = pp.tile([128, T], F32, name="big", bufs=2)[:, 0:128]
            nc.tensor.transpose(trp, aq_rows[:, hc * 128:(hc + 1) * 128], ident)
            nc.scalar.activation(aq_s[:, hc * 128:(hc + 1) * 128], trp, AF.Copy)
            trp2 = pp.tile([128, T], BF16, name="bigb", bufs=1)[:, 0:128]
            nc.tensor.transpose(trp2, oq_rows[:, hc * 128:(hc + 1) * 128], identb)
            nc.scalar.activation(oqT_s[:, hc * 128:(hc + 1) * 128], trp2, AF.Copy)

        bT_s = cp.tile([128, 2 * T], BF16)  # col = hc*512 + j
        our = [cp.tile([128, H], BF16, name=f"our{jc}") for jc in range(4)]
        for jc in range(4):
            b_rows = sp.tile([128, H], F32, name="b_rows", bufs=4)
            nc.gpsimd.indirect_dma_start(
                out=b_rows, out_offset=None, in_=b_dram[:],
                in_offset=bass.IndirectOffsetOnAxis(ap=ki_s[:, jc:jc + 1], axis=0))
            nc.gpsimd.indirect_dma_start(
                out=our[jc], out_offset=None, in_=o_dram[:],
                in_offset=bass.IndirectOffsetOnAxis(ap=ki_s[:, jc:jc + 1], axis=0))
            for hc in range(2):
                trp = pp.tile([128, T], F32, name="big", bufs=2)[:, 0:128]
                nc.tensor.transpose(trp, b_rows[:, hc * 128:(hc + 1) * 128], ident)
                nc.scalar.activation(
                    bT_s[:, hc * T + jc * 128: hc * T + (jc + 1) * 128], trp, AF.Copy)

        # ---- scores + softmax ----
        sm_s = cp.tile([128, T], F32)
        for q in range(QB):
            sc1 = pp.tile([1, T], F32, name="sc1", bufs=2)
            for hc in range(2):
                th = sp.tile([128, T], BF16, name="th", bufs=4)
                nc.scalar.activation(
                    th, bT_s[:, hc * T:(hc + 1) * T], AF.Tanh,
                    bias=aq_s[:, hc * 128 + q: hc * 128 + q + 1])
                nc.tensor.matmul(sc1, vt_s[:, hc:hc + 1], th,
                                 start=(hc == 0), stop=(hc == 1))
            scq = sp.tile([1, T], F32, name="scq", bufs=4)
            nc.vector.tensor_copy(out=scq, in_=sc1)
            nc.sync.dma_start(out=sm_s[q:q + 1, :], in_=scq)

        # ---- projection, oq half: emitted after the scores loop so it
        # backfills PE idle slots during the (ACT-bound) scores phase;
        # partial logits staged in SBUF as bf16 ----
        partial = cp.tile([128, NVB * VB], BF16)
        for vb in range(NVB):
            wt1 = wp.tile([128, 2 * VB], BF16, name="wt1")
            nc.sync.dma_start(
                out=wt1[:].rearrange("p (a v) -> p a v", a=2),
                in_=wfc_e[0:256, vb * VB:(vb + 1) * VB].rearrange(
                    "(a p) v -> p a v", p=128))
            ps = pp.tile([128, T], F32, name="big", bufs=2)[:, 0:VB]
            for kc in range(2):
                nc.tensor.matmul(ps, oqT_s[:, kc * 128:(kc + 1) * 128],
                                 wt1[:, kc * VB:(kc + 1) * VB],
                                 start=(kc == 0), stop=(kc == 1))
            nc.vector.tensor_copy(out=partial[:, vb * VB:(vb + 1) * VB], in_=ps)

        nc.vector.tensor_add(out=sm_s, in0=sm_s, in1=mask_s)
        nmx = cp.tile([128, 1], F32)
        nc.vector.reduce_max(nmx, sm_s, axis=AX.X, negate=True)
        ex_s = cp.tile([128, T], F32)
        ssum = cp.tile([128, 1], F32)
        nc.scalar.activation(ex_s, sm_s, AF.Exp, bias=nmx, accum_out=ssum)
        rs = cp.tile([128, 1], F32)
        nc.vector.reciprocal(rs, ssum)
        at_s = cp.tile([128, T], F32)
        nc.vector.tensor_scalar(out=at_s, in0=ex_s, scalar1=rs, scalar2=None,
                                op0=mybir.AluOpType.mult)

        # ---- context: ctxT [h, q] ----
        ctx_ps = pp.tile([128, T], F32, name="big", bufs=2)[:, 0:H]
        atT = [cp.tile([128, 128], BF16, name=f"atT{jc}") for jc in range(4)]
        for jc in range(4):
            trp = pp.tile([128, T], F32, name="big", bufs=2)[:, 0:128]
            nc.tensor.transpose(trp, at_s[:, jc * 128:(jc + 1) * 128], ident)
            nc.scalar.activation(atT[jc], trp, AF.Copy)
        for hc in range(2):
            for jc in range(4):
                nc.tensor.matmul(ctx_ps[:, hc * 128:(hc + 1) * 128],
                                 our[jc][:, hc * 128:(hc + 1) * 128], atT[jc],
                                 start=(jc == 0), stop=(jc == 3))
        ctxT_s = cp.tile([128, H], BF16)
        nc.vector.tensor_copy(out=ctxT_s, in_=ctx_ps)

        # ---- projection, ctx half + staged oq partial ----
        for vb in range(NVB):
            wt2 = wp.tile([128, 2 * VB], BF16, name="wt2")
            nc.sync.dma_start(
                out=wt2[:].rearrange("p (a v) -> p a v", a=2),
                in_=wfc_e[256:512, vb * VB:(vb + 1) * VB].rearrange(
                    "(a p) v -> p a v", p=128))
            # alternate between two PSUM tags (4 banks total) for a deeper
            # matmul/add/store pipeline
            lg_ps = pp.tile([128, T], F32, name=("big" if vb % 2 else "sc1"),
                            bufs=2)[:, 0:VB]
            for kc in range(2):
                nc.tensor.matmul(lg_ps, ctxT_s[:, kc * 128:(kc + 1) * 128],
                                 wt2[:, kc * VB:(kc + 1) * VB],
                                 start=(kc == 0), stop=(kc == 1))
            lg_sb = sp.tile([128, VB], F32, name="lg_sb", bufs=4)
            nc.vector.tensor_add(out=lg_sb, in0=lg_ps,
                                 in1=partial[:, vb * VB:(vb + 1) * VB])
            nc.sync.dma_start(out=out_e[:, vb * VB:(vb + 1) * VB], in_=lg_sb)

    nc.finalize()
    return nc


_NC = None


def _get_nc():
    global _NC
    if _NC is None:
        _NC = build()
    return _NC


def _prep(inputs):
    x = np.asarray(inputs["x"])
    perm = np.concatenate([np.arange(0, 512), np.arange(768, 1024),
                           np.arange(512, 768)])
    wihT = np.ascontiguousarray(np.asarray(inputs["W_ih"])[perm].T.astype(BFNP))
    whhT = np.ascontiguousarray(np.asarray(inputs["W_hh"])[perm].T.astype(BFNP))
    bias = (np.asarray(inputs["b_ih"]) + np.asarray(inputs["b_hh"]))[perm]
    biasT = np.ascontiguousarray(bias.reshape(8, 128).T)
    w1T = np.ascontiguousarray(np.asarray(inputs["W1"]).T.astype(BFNP))
    w2T = np.ascontiguousarray(np.asarray(inputs["W2"]).T.astype(BFNP))
    b12 = (np.asarray(inputs["b1"]) + np.asarray(inputs["b2"])).reshape(1, H)
    vt = np.ascontiguousarray(np.asarray(inputs["V"])[0].reshape(2, 128).T.astype(BFNP))
    wfcT = np.ascontiguousarray(np.asarray(inputs["Wfc"]).T.astype(BFNP))
    xt = np.zeros((128, 8), np.int32)
    for b in range(B):
        for tch in range(4):
            xt[:, b * 4 + tch] = x[b, tch * 128:(tch + 1) * 128]
    common = dict(
        emb=np.ascontiguousarray(np.asarray(inputs["emb"], np.float32)),
        xt=xt, wihT=wihT, whhT=whhT,
        biasT=np.ascontiguousarray(biasT.astype(np.float32)),
        w1T=w1T, w2T=w2T,
        b12=np.ascontiguousarray(b12.astype(BFNP)), vt=vt,
        wfcT=wfcT)
    r = np.arange(128)
    in_maps = []
    for c in range(NCORES):
        b, ib = divmod(c, 4)
        qi = (b * T + ib * 128 + r).astype(np.int32).reshape(128, 1)
        ki = np.stack([(b * T + jc * 128 + r).astype(np.int32)
                       for jc in range(4)], axis=1)
        mask = np.where(np.arange(T)[None, :] <= (ib * 128 + r)[:, None],
                        np.float32(0.0), np.float32(-1e30)).astype(np.float32)
        m = dict(common)
        m.update(qi=qi, ki=np.ascontiguousarray(ki), mask=mask)
        in_maps.append(m)
    return in_maps


LAST = None


def assemble(results, inputs):
    bfc = np.asarray(inputs["bfc"], np.float32)
    logits = np.empty((B, T, VOCAB), np.float32)
    for c in range(NCORES):
        b, ib = divmod(c, 4)
        logits[b, ib * 128:(ib + 1) * 128, :] = results[c]["out"]
    logits += bfc[None, None, :]
    return logits


def kernel(**inputs):
    global LAST
    nc = _get_nc()
    in_maps = _prep(inputs)
    trace = bool(os.environ.get("KERNEL_TRACE"))
    try:
        br = run_bass_kernel_spmd(nc, in_maps, list(range(NCORES)), trace=trace)
    except Exception:
        if not trace:
            raise
        br = run_bass_kernel_spmd(nc, in_maps, list(range(NCORES)), trace=False)
    LAST = br
    return assemble(br.results, inputs)


if __name__ == "__main__":
    build()
    print("build ok")


# revision 30
# speedup vs baseline: 1.2511x; 1.0287x over previous
"""RNN(LSTM)+additive-attention language model on 8 trn2 cores.

Sharding: every core runs the full LSTM (both batches, merged into one set of
[128, 4] state tiles); core c = (b, ib) then does attention + vocab projection
for query rows [ib*128, (ib+1)*128) of batch b. Per-core row selection is done
with indirect-DMA gathers driven by per-core int32 index inputs, so all 8
cores run one identical SPMD program.

All matmuls run in bf16 (fp32 matmuls double-pump as HIGH/LOW instruction
pairs and disable fast-weight-load, which made LDWEIGHTS the kernel
bottleneck). Accumulation stays fp32 in PSUM; the softmax and LSTM cell
state stay fp32.
"""

import os
import numpy as np
import ml_dtypes
from contextlib import ExitStack

import concourse.bass as bass
import concourse.tile as tile
from concourse import bacc, mybir
from concourse.bass_utils import run_bass_kernel_spmd
from concourse.masks import make_identity

F32 = mybir.dt.float32
BF16 = mybir.dt.bfloat16
I32 = mybir.dt.int32
AF = mybir.ActivationFunctionType
AX = mybir.AxisListType
BFNP = ml_dtypes.bfloat16

B, T, E, H, VOCAB = 2, 512, 256, 256, 32000
NCORES = 8
QB = 128          # query rows per core
VB = 500          # vocab cols per projection block
NVB = VOCAB // VB  # 64


def build():
    nc = bacc.Bacc("TRN2", num_devices=NCORES)

    emb_e = nc.declare_dram_parameter("emb", [VOCAB, E], F32, isOutput=False)
    xt_e = nc.declare_dram_parameter("xt", [128, 8], I32, isOutput=False)
    wih_e = nc.declare_dram_parameter("wihT", [E, 4 * H], BF16, isOutput=False)
    whh_e = nc.declare_dram_parameter("whhT", [H, 4 * H], BF16, isOutput=False)
    bT_e = nc.declare_dram_parameter("biasT", [128, 8], F32, isOutput=False)
    w1_e = nc.declare_dram_parameter("w1T", [H, H], BF16, isOutput=False)
    w2_e = nc.declare_dram_parameter("w2T", [H, H], BF16, isOutput=False)
    b12_e = nc.declare_dram_parameter("b12", [1, H], BF16, isOutput=False)
    vt_e = nc.declare_dram_parameter("vt", [128, 2], BF16, isOutput=False)
    wfc_e = nc.declare_dram_parameter("wfcT", [2 * H, VOCAB], BF16, isOutput=False)
    qi_e = nc.declare_dram_parameter("qi", [128, 1], I32, isOutput=False)
    ki_e = nc.declare_dram_parameter("ki", [128, 4], I32, isOutput=False)
    mask_e = nc.declare_dram_parameter("mask", [128, T], F32, isOutput=False)
    out_e = nc.declare_dram_parameter("out", [QB, VOCAB], F32, isOutput=True)

    a_dram = nc.dram_tensor("a_scr", [B * T, H], F32)
    o_dram = nc.dram_tensor("o_scr", [B * T, H], BF16)
    # per-j-chunk scratch so key-side gathers only depend on their own
    # chunk's writes and can run during the LSTM
    b_dram_c = [nc.dram_tensor(f"b_scr{j}", [B * 128, H], F32) for j in range(4)]
    o_dram_c = [nc.dram_tensor(f"o_scr{j}", [B * 128, H], BF16) for j in range(4)]

    with tile.TileContext(nc) as tc, ExitStack() as ctx:
        cp = ctx.enter_context(tc.tile_pool(name="cp", bufs=1))
        sp = ctx.enter_context(tc.tile_pool(name="sp", bufs=3))
        wp = ctx.enter_context(tc.tile_pool(name="wp", bufs=8))
        pp = ctx.enter_context(tc.tile_pool(name="pp", bufs=2, space="PSUM"))

        # ---- constants / params ----
        ident = cp.tile([128, 128], F32)
        make_identity(nc, ident)
        identb = cp.tile([128, 128], BF16)
        nc.vector.tensor_copy(out=identb, in_=ident)
        ones_s = cp.tile([1, 128], BF16)
        nc.vector.memset(ones_s, 1.0)

        wih_s = cp.tile([128, 2 * 4 * H], BF16)   # col = kc*1024 + g
        whh_s = cp.tile([128, 2 * 4 * H], BF16)
        for kc in range(2):
            nc.sync.dma_start(out=wih_s[:, kc * 1024:(kc + 1) * 1024],
                              in_=wih_e[kc * 128:(kc + 1) * 128, :])
            nc.sync.dma_start(out=whh_s[:, kc * 1024:(kc + 1) * 1024],
                              in_=whh_e[kc * 128:(kc + 1) * 128, :])
        biasT_s = cp.tile([128, 8], F32)
        nc.sync.dma_start(out=biasT_s, in_=bT_e[:])
        w1_s = cp.tile([128, 2 * H], BF16)        # col = hcin*256 + hout
        w2_s = cp.tile([128, 2 * H], BF16)
        for kc in range(2):
            nc.sync.dma_start(out=w1_s[:, kc * H:(kc + 1) * H],
                              in_=w1_e[kc * 128:(kc + 1) * 128, :])
            nc.sync.dma_start(out=w2_s[:, kc * H:(kc + 1) * H],
                              in_=w2_e[kc * 128:(kc + 1) * 128, :])
        b12_s = cp.tile([1, H], BF16)
        nc.sync.dma_start(out=b12_s, in_=b12_e[:])
        vt_s = cp.tile([128, 2], BF16)
        nc.sync.dma_start(out=vt_s, in_=vt_e[:])
        xt_s = cp.tile([128, 8], I32)
        nc.sync.dma_start(out=xt_s, in_=xt_e[:])
        qi_s = cp.tile([128, 1], I32)
        nc.sync.dma_start(out=qi_s, in_=qi_e[:])
        ki_s = cp.tile([128, 4], I32)
        nc.sync.dma_start(out=ki_s, in_=ki_e[:])
        mask_s = cp.tile([128, T], F32)
        nc.sync.dma_start(out=mask_s, in_=mask_e[:])

        # ---- embedding gather + transpose -> xeT[b] [128, 2ec*512] bf16 ----
        xeT = [cp.tile([128, 2 * T], BF16, name=f"xeT{b}") for b in range(B)]
        for tch in range(4):
            for b in range(B):
                xe_rows = sp.tile([128, E], F32)
                nc.gpsimd.indirect_dma_start(
                    out=xe_rows, out_offset=None, in_=emb_e[:],
                    in_offset=bass.IndirectOffsetOnAxis(
                        ap=xt_s[:, b * 4 + tch:b * 4 + tch + 1], axis=0))
                for ec in range(2):
                    trp = pp.tile([128, T], F32, name="big", bufs=2)[:, 0:128]
                    nc.tensor.transpose(trp, xe_rows[:, ec * 128:(ec + 1) * 128], ident)
                    nc.scalar.activation(
                        xeT[b][:, ec * T + tch * 128: ec * T + (tch + 1) * 128],
                        trp, AF.Copy)

        # ---- gx precompute: gxT [128, T*16] bf16, col = t*16 + gc*2 + b ----
        # chunked over t so the LSTM can start after the first chunk
        gxT = cp.tile([128, T * 16], BF16)
        TCH = 64
        for tch in range(T // TCH):
            for b in range(B):
                for gc in range(8):
                    gx_ps = pp.tile([128, T], F32, name="big", bufs=2)[:, 0:TCH]
                    for ec in range(2):
                        nc.tensor.matmul(
                            gx_ps,
                            wih_s[:, ec * 1024 + gc * 128: ec * 1024 + (gc + 1) * 128],
                            xeT[b][:, ec * T + tch * TCH: ec * T + (tch + 1) * TCH],
                            start=(ec == 0), stop=(ec == 1))
                    off = tch * TCH * 16 + gc * 2 + b
                    nc.vector.tensor_scalar(
                        out=gxT[:, off: off + 16 * (TCH - 1) + 1: 16], in0=gx_ps,
                        scalar1=biasT_s[:, gc:gc + 1], scalar2=None,
                        op0=mybir.AluOpType.add)

        # ---- LSTM ----
        # outT_all col = t*4 + kc*2 + b (h in bf16, written directly by the
        # h-mul). act tiles (ping-pong): cols 0:4=i, 4:8=f, 8:12=o,
        # 12:16=tanh(g), 16:20=c_prev (f32). gx is injected into PSUM with an
        # identity matmul so the gate activations read PSUM directly.
        # g-gate matmuls run first so tanh(g) overlaps the remaining matmuls.
        outT_all = cp.tile([128, 4 * T], BF16)
        acts = [cp.tile([128, 20], F32, name=f"act{i}") for i in range(2)]
        nc.vector.memset(acts[0][:, 16:20], 0.0)
        z4 = cp.tile([128, 4], BF16)
        nc.vector.memset(z4, 0.0)
        # gate order in tiles (host perm): i, f, o, g -> gc blocks
        #   i: gc0,1  f: gc2,3  o: gc4,5  g: gc6,7
        # gates go to three separate PSUM tiles (banks) so each activation
        # only waits for its own gate matmuls (PSUM deps are bank-level):
        # g first (tanh overlaps i/f matmuls), then i,f, then o.
        MM_ORDER = [6, 7, 0, 1, 2, 3, 4, 5]

        def outv(b, hc, t0, n):
            """[128, n] bf16 view of outputs: h-chunk hc, batch b, t0..t0+n."""
            s = t0 * 4 + hc * 2 + b
            return outT_all[:, s: s + 4 * (n - 1) + 1: 4]

        bT_s = cp.tile([128, 2 * T], BF16)  # col = hc*512 + j
        our = [cp.tile([128, H], BF16, name=f"our{jc}") for jc in range(4)]

        def features(tch):
            """a/b features + output rows -> DRAM for t-chunk tch, then the
            key-side gathers/transposes for that chunk.

            Emitted right after the LSTM steps that produce chunk tch, so the
            scheduler can backfill everything into LSTM idle slots.
            """
            for b in range(B):
                for w_s, dram, row0, with_bias in (
                        (w1_s, a_dram, b * T + tch * 128, True),
                        (w2_s, b_dram_c[tch], b * 128, False)):
                    f_t = pp.tile([128, T], F32, name="big", bufs=2)
                    f_ps = f_t[:, 0:H]
                    for hc in range(2):
                        nc.tensor.matmul(
                            f_ps,
                            outv(b, hc, tch * 128, 128),
                            w_s[:, hc * H:(hc + 1) * H],
                            start=(hc == 0),
                            stop=(False if with_bias else hc == 1))
                    if with_bias:
                        nc.tensor.matmul(f_ps, ones_s, b12_s, start=False, stop=True)
                    f_sb = sp.tile([128, H], F32, name="f_sb", bufs=4)
                    nc.vector.tensor_copy(out=f_sb, in_=f_ps)
                    nc.sync.dma_start(out=dram[row0: row0 + 128, :], in_=f_sb)
                o_sb = sp.tile([128, H], BF16, name="o_sb", bufs=4)
                for hc in range(2):
                    trp = pp.tile([128, T], BF16, name="bigb", bufs=1)[:, 0:128]
                    nc.tensor.transpose(
                        trp, outv(b, hc, tch * 128, 128), identb)
                    nc.scalar.activation(o_sb[:, hc * 128:(hc + 1) * 128], trp, AF.Copy)
                nc.sync.dma_start(
                    out=o_dram[b * T + tch * 128: b * T + (tch + 1) * 128, :],
                    in_=o_sb)
                nc.sync.dma_start(
                    out=o_dram_c[tch][b * 128:(b + 1) * 128, :], in_=o_sb)
            # key-side gathers for this chunk (per-core batch pick via ki)
            b_rows = sp.tile([128, H], F32, name="b_rows", bufs=4)
            nc.gpsimd.indirect_dma_start(
                out=b_rows, out_offset=None, in_=b_dram_c[tch][:],
                in_offset=bass.IndirectOffsetOnAxis(ap=ki_s[:, tch:tch + 1], axis=0))
            nc.gpsimd.indirect_dma_start(
                out=our[tch], out_offset=None, in_=o_dram_c[tch][:],
                in_offset=bass.IndirectOffsetOnAxis(ap=ki_s[:, tch:tch + 1], axis=0))
            for hc in range(2):
                trp = pp.tile([128, T], F32, name="big", bufs=2)[:, 0:128]
                nc.tensor.transpose(trp, b_rows[:, hc * 128:(hc + 1) * 128], ident)
                nc.scalar.activation(
                    bT_s[:, hc * T + tch * 128: hc * T + (tch + 1) * 128],
                    trp, AF.Copy)

        for t in range(T):
            A = acts[t % 2]
            An = acts[(t + 1) % 2]
            hT = z4 if t == 0 else outT_all[:, (t - 1) * 4: t * 4]
            g_if = pp.tile([128, 8], F32, name="gps_if", bufs=1)
            g_o = pp.tile([128, 4], F32, name="gps_o", bufs=1)
            g_g = pp.tile([128, 4], F32, name="gps_g", bufs=1)

            def gview(gc):
                if gc < 4:
                    return g_if[:, gc * 2: gc * 2 + 2]
                if gc < 6:
                    return g_o[:, (gc - 4) * 2: (gc - 4) * 2 + 2]
                return g_g[:, (gc - 6) * 2: (gc - 6) * 2 + 2]

            nc.tensor.matmul(g_g, identb, gxT[:, t * 16 + 12: t * 16 + 16],
                             start=True, stop=False, skip_group_check=True)
            nc.tensor.matmul(g_if, identb, gxT[:, t * 16: t * 16 + 8],
                             start=True, stop=False, skip_group_check=True)
            nc.tensor.matmul(g_o, identb, gxT[:, t * 16 + 8: t * 16 + 12],
                             start=True, stop=False, skip_group_check=True)
            for i, gc in enumerate(MM_ORDER):
                for kc in range(2):
                    nc.tensor.matmul(
                        gview(gc),
                        whh_s[:, kc * 1024 + gc * 128: kc * 1024 + (gc + 1) * 128],
                        hT[:, kc * 2: kc * 2 + 2],
                        start=False, stop=(i == 7 and kc == 1),
                        skip_group_check=True)
            nc.scalar.activation(A[:, 12:16], g_g, AF.Tanh)
            nc.scalar.activation(A[:, 0:8], g_if, AF.Sigmoid)
            nc.scalar.activation(A[:, 8:12], g_o, AF.Sigmoid)
            prod = sp.tile([128, 8], F32, name="prod")
            nc.vector.tensor_mul(out=prod, in0=A[:, 0:8], in1=A[:, 12:20])
            nc.vector.tensor_add(out=An[:, 16:20], in0=prod[:, 0:4],
                                 in1=prod[:, 4:8])
            thc = sp.tile([128, 4], F32, name="thc")
            nc.scalar.activation(thc, An[:, 16:20], AF.Tanh)
            nc.vector.tensor_mul(out=outT_all[:, t * 4:(t + 1) * 4],
                                 in0=A[:, 8:12], in1=thc)
            if (t + 1) % 128 == 0:
                features((t + 1) // 128 - 1)

        # ---- gathers for this core's (b, iblock) ----
        aq_rows = cp.tile([128, H], F32)
        nc.gpsimd.indirect_dma_start(
            out=aq_rows, out_offset=None, in_=a_dram[:],
            in_offset=bass.IndirectOffsetOnAxis(ap=qi_s[:, 0:1], axis=0))
        oq_rows = cp.tile([128, H], BF16)
        nc.gpsimd.indirect_dma_start(
            out=oq_rows, out_offset=None, in_=o_dram[:],
            in_offset=bass.IndirectOffsetOnAxis(ap=qi_s[:, 0:1], axis=0))
        aq_s = cp.tile([128, H], BF16)    # col = hc*128 + q
        oqT_s = cp.tile([128, H], BF16)
        for hc in range(2):
            trp = pp.tile([128, T], F32, name="big", bufs=2)[:, 0:128]
            nc.tensor.transpose(trp, aq_rows[:, hc * 128:(hc + 1) * 128], ident)
            nc.scalar.activation(aq_s[:, hc * 128:(hc + 1) * 128], trp, AF.Copy)
            trp2 = pp.tile([128, T], BF16, name="bigb", bufs=1)[:, 0:128]
            nc.tensor.transpose(trp2, oq_rows[:, hc * 128:(hc + 1) * 128], identb)
            nc.scalar.activation(oqT_s[:, hc * 128:(hc + 1) * 128], trp2, AF.Copy)

        # ---- scores + softmax ----
        sm_s = cp.tile([128, T], F32)
        for q in range(QB):
            sc1 = pp.tile([1, T], F32, name="sc1", bufs=2)
            for hc in range(2):
                th = sp.tile([128, T], BF16, name="th", bufs=4)
                nc.scalar.activation(
                    th, bT_s[:, hc * T:(hc + 1) * T], AF.Tanh,
                    bias=aq_s[:, hc * 128 + q: hc * 128 + q + 1])
                nc.tensor.matmul(sc1, vt_s[:, hc:hc + 1], th,
                                 start=(hc == 0), stop=(hc == 1))
            scq = sp.tile([1, T], F32, name="scq", bufs=4)
            nc.vector.tensor_copy(out=scq, in_=sc1)
            nc.sync.dma_start(out=sm_s[q:q + 1, :], in_=scq)

        # ---- projection, oq half: emitted after the scores loop so it
        # backfills PE idle slots during the (ACT-bound) scores phase;
        # partial logits staged in SBUF as bf16 ----
        partial = cp.tile([128, NVB * VB], BF16)
        for vb in range(NVB):
            wt1 = wp.tile([128, 2 * VB], BF16, name="wt1")
            nc.sync.dma_start(
                out=wt1[:].rearrange("p (a v) -> p a v", a=2),
                in_=wfc_e[0:256, vb * VB:(vb + 1) * VB].rearrange(
                    "(a p) v -> p a v", p=128))
            ps = pp.tile([128, T], F32, name="big", bufs=2)[:, 0:VB]
            for kc in range(2):
                nc.tensor.matmul(ps, oqT_s[:, kc * 128:(kc + 1) * 128],
                                 wt1[:, kc * VB:(kc + 1) * VB],
                                 start=(kc == 0), stop=(kc == 1))
            nc.vector.tensor_copy(out=partial[:, vb * VB:(vb + 1) * VB], in_=ps)

        nc.vector.tensor_add(out=sm_s, in0=sm_s, in1=mask_s)
        nmx = cp.tile([128, 1], F32)
        nc.vector.reduce_max(nmx, sm_s, axis=AX.X, negate=True)
        ex_s = cp.tile([128, T], F32)
        ssum = cp.tile([128, 1], F32)
        nc.scalar.activation(ex_s, sm_s, AF.Exp, bias=nmx, accum_out=ssum)
        rs = cp.tile([128, 1], F32)
        nc.vector.reciprocal(rs, ssum)
        at_s = cp.tile([128, T], F32)
        nc.vector.tensor_scalar(out=at_s, in0=ex_s, scalar1=rs, scalar2=None,
                                op0=mybir.AluOpType.mult)

        # ---- context: ctxT [h, q] ----
        ctx_ps = pp.tile([128, T], F32, name="big", bufs=2)[:, 0:H]
        atT = [cp.tile([128, 128], BF16, name=f"atT{jc}") for jc in range(4)]
        for jc in range(4):
            trp = pp.tile([128, T], F32, name="big", bufs=2)[:, 0:128]
            nc.tensor.transpose(trp, at_s[:, jc * 128:(jc + 1) * 128], ident)
            nc.scalar.activation(atT[jc], trp, AF.Copy)
        for hc in range(2):
            for jc in range(4):
                nc.tensor.matmul(ctx_ps[:, hc * 128:(hc + 1) * 128],
                                 our[jc][:, hc * 128:(hc + 1) * 128], atT[jc],
                                 start=(jc == 0), stop=(jc == 3))
        ctxT_s = cp.tile([128, H], BF16)
        nc.vector.tensor_copy(out=ctxT_s, in_=ctx_ps)

        # ---- projection, ctx half + staged oq partial ----
        for vb in range(NVB):
            wt2 = wp.tile([128, 2 * VB], BF16, name="wt2")
            nc.scalar.dma_start(
                out=wt2[:].rearrange("p (a v) -> p a v", a=2),
                in_=wfc_e[256:512, vb * VB:(vb + 1) * VB].rearrange(
                    "(a p) v -> p a v", p=128))
            # alternate between two PSUM tags (4 banks total) for a deeper
            # matmul/add/store pipeline
            lg_ps = pp.tile([128, T], F32, name=("big" if vb % 2 else "sc1"),
                            bufs=2)[:, 0:VB]
            for kc in range(2):
                nc.tensor.matmul(lg_ps, ctxT_s[:, kc * 128:(kc + 1) * 128],
                                 wt2[:, kc * VB:(kc + 1) * VB],
                                 start=(kc == 0), stop=(kc == 1))
            lg_sb = sp.tile([128, VB], F32, name="lg_sb", bufs=4)
            nc.vector.tensor_add(out=lg_sb, in0=lg_ps,
                                 in1=partial[:, vb * VB:(vb + 1) * VB])
            nc.sync.dma_start(out=out_e[:, vb * VB:(vb + 1) * VB], in_=lg_sb)

    nc.finalize()
    return nc


_NC = None


def _get_nc():
    global _NC
    if _NC is None:
        _NC = build()
    return _NC


def _prep(inputs):
    x = np.asarray(inputs["x"])
    perm = np.concatenate([np.arange(0, 512), np.arange(768, 1024),
                           np.arange(512, 768)])
    wihT = np.ascontiguousarray(np.asarray(inputs["W_ih"])[perm].T.astype(BFNP))
    whhT = np.ascontiguousarray(np.asarray(inputs["W_hh"])[perm].T.astype(BFNP))
    bias = (np.asarray(inputs["b_ih"]) + np.asarray(inputs["b_hh"]))[perm]
    biasT = np.ascontiguousarray(bias.reshape(8, 128).T)
    w1T = np.ascontiguousarray(np.asarray(inputs["W1"]).T.astype(BFNP))
    w2T = np.ascontiguousarray(np.asarray(inputs["W2"]).T.astype(BFNP))
    b12 = (np.asarray(inputs["b1"]) + np.asarray(inputs["b2"])).reshape(1, H)
    vt = np.ascontiguousarray(np.asarray(inputs["V"])[0].reshape(2, 128).T.astype(BFNP))
    wfcT = np.ascontiguousarray(np.asarray(inputs["Wfc"]).T.astype(BFNP))
    xt = np.zeros((128, 8), np.int32)
    for b in range(B):
        for tch in range(4):
            xt[:, b * 4 + tch] = x[b, tch * 128:(tch + 1) * 128]
    common = dict(
        emb=np.ascontiguousarray(np.asarray(inputs["emb"], np.float32)),
        xt=xt, wihT=wihT, whhT=whhT,
        biasT=np.ascontiguousarray(biasT.astype(np.float32)),
        w1T=w1T, w2T=w2T,
        b12=np.ascontiguousarray(b12.astype(BFNP)), vt=vt,
        wfcT=wfcT)
    r = np.arange(128)
    in_maps = []
    for c in range(NCORES):
        b, ib = divmod(c, 4)
        qi = (b * T + ib * 128 + r).astype(np.int32).reshape(128, 1)
        ki = np.stack([(b * 128 + r).astype(np.int32)
                       for jc in range(4)], axis=1)
        mask = np.where(np.arange(T)[None, :] <= (ib * 128 + r)[:, None],
                        np.float32(0.0), np.float32(-1e30)).astype(np.float32)
        m = dict(common)
        m.update(qi=qi, ki=np.ascontiguousarray(ki), mask=mask)
        in_maps.append(m)
    return in_maps


LAST = None


def assemble(results, inputs):
    bfc = np.asarray(inputs["bfc"], np.float32)
    logits = np.empty((B, T, VOCAB), np.float32)
    for c in range(NCORES):
        b, ib = divmod(c, 4)
        logits[b, ib * 128:(ib + 1) * 128, :] = results[c]["out"]
    logits += bfc[None, None, :]
    return logits


def kernel(**inputs):
    global LAST
    nc = _get_nc()
    in_maps = _prep(inputs)
    trace = bool(os.environ.get("KERNEL_TRACE"))
    try:
        br = run_bass_kernel_spmd(nc, in_maps, list(range(NCORES)), trace=trace)
    except Exception:
        if not trace:
            raise
        br = run_bass_kernel_spmd(nc, in_maps, list(range(NCORES)), trace=False)
    LAST = br
    return assemble(br.results, inputs)


if __name__ == "__main__":
    build()
    print("build ok")


# revision 33
# speedup vs baseline: 1.2690x; 1.0144x over previous
"""RNN(LSTM)+additive-attention language model on 8 trn2 cores.

Sharding: every core runs the full LSTM (both batches, merged into one set of
[128, 4] state tiles); core c = (b, ib) then does attention + vocab projection
for query rows [ib*128, (ib+1)*128) of batch b. Per-core row selection is done
with indirect-DMA gathers driven by per-core int32 index inputs, so all 8
cores run one identical SPMD program.

All matmuls run in bf16 (fp32 matmuls double-pump as HIGH/LOW instruction
pairs and disable fast-weight-load, which made LDWEIGHTS the kernel
bottleneck). Accumulation stays fp32 in PSUM; the softmax and LSTM cell
state stay fp32.
"""

import os
import numpy as np
import ml_dtypes
from contextlib import ExitStack

import concourse.bass as bass
import concourse.tile as tile
from concourse import bacc, mybir
from concourse.bass_utils import run_bass_kernel_spmd
from concourse.masks import make_identity

F32 = mybir.dt.float32
BF16 = mybir.dt.bfloat16
I32 = mybir.dt.int32
AF = mybir.ActivationFunctionType
AX = mybir.AxisListType
BFNP = ml_dtypes.bfloat16

B, T, E, H, VOCAB = 2, 512, 256, 256, 32000
NCORES = 8
QB = 128          # query rows per core
VB = 500          # vocab cols per projection block
NVB = VOCAB // VB  # 64


def build():
    nc = bacc.Bacc("TRN2", num_devices=NCORES)

    emb_e = nc.declare_dram_parameter("emb", [VOCAB, E], F32, isOutput=False)
    xt_e = nc.declare_dram_parameter("xt", [128, 8], I32, isOutput=False)
    wih_e = nc.declare_dram_parameter("wihT", [E, 4 * H], BF16, isOutput=False)
    whh_e = nc.declare_dram_parameter("whhT", [H, 4 * H], BF16, isOutput=False)
    bT_e = nc.declare_dram_parameter("biasT", [128, 8], F32, isOutput=False)
    w1_e = nc.declare_dram_parameter("w1T", [H, H], BF16, isOutput=False)
    w2_e = nc.declare_dram_parameter("w2T", [H, H], BF16, isOutput=False)
    b12_e = nc.declare_dram_parameter("b12", [1, H], BF16, isOutput=False)
    vt_e = nc.declare_dram_parameter("vt", [128, 2], BF16, isOutput=False)
    wfc_e = nc.declare_dram_parameter("wfcT", [2 * H, VOCAB], BF16, isOutput=False)
    qi_e = nc.declare_dram_parameter("qi", [128, 1], I32, isOutput=False)
    ki_e = nc.declare_dram_parameter("ki", [128, 4], I32, isOutput=False)
    mask_e = nc.declare_dram_parameter("mask", [128, T], F32, isOutput=False)
    out_e = nc.declare_dram_parameter("out", [QB, VOCAB], F32, isOutput=True)

    a_dram = nc.dram_tensor("a_scr", [B * T, H], F32)
    o_dram = nc.dram_tensor("o_scr", [B * T, H], BF16)
    # per-j-chunk scratch so key-side gathers only depend on their own
    # chunk's writes and can run during the LSTM
    b_dram_c = [nc.dram_tensor(f"b_scr{j}", [B * 128, H], F32) for j in range(4)]
    o_dram_c = [nc.dram_tensor(f"o_scr{j}", [B * 128, H], BF16) for j in range(4)]

    with tile.TileContext(nc) as tc, ExitStack() as ctx:
        cp = ctx.enter_context(tc.tile_pool(name="cp", bufs=1))
        sp = ctx.enter_context(tc.tile_pool(name="sp", bufs=3))
        wp = ctx.enter_context(tc.tile_pool(name="wp", bufs=8))
        pp = ctx.enter_context(tc.tile_pool(name="pp", bufs=2, space="PSUM"))

        # ---- constants / params ----
        ident = cp.tile([128, 128], F32)
        make_identity(nc, ident)
        identb = cp.tile([128, 128], BF16)
        nc.vector.tensor_copy(out=identb, in_=ident)
        ones_s = cp.tile([1, 128], BF16)
        nc.vector.memset(ones_s, 1.0)

        wih_s = cp.tile([128, 2 * 4 * H], BF16)   # col = kc*1024 + g
        whh_s = cp.tile([128, 2 * 4 * H], BF16)
        for kc in range(2):
            nc.sync.dma_start(out=wih_s[:, kc * 1024:(kc + 1) * 1024],
                              in_=wih_e[kc * 128:(kc + 1) * 128, :])
            nc.sync.dma_start(out=whh_s[:, kc * 1024:(kc + 1) * 1024],
                              in_=whh_e[kc * 128:(kc + 1) * 128, :])
        biasT_s = cp.tile([128, 8], F32)
        nc.sync.dma_start(out=biasT_s, in_=bT_e[:])
        w1_s = cp.tile([128, 2 * H], BF16)        # col = hcin*256 + hout
        w2_s = cp.tile([128, 2 * H], BF16)
        for kc in range(2):
            nc.sync.dma_start(out=w1_s[:, kc * H:(kc + 1) * H],
                              in_=w1_e[kc * 128:(kc + 1) * 128, :])
            nc.sync.dma_start(out=w2_s[:, kc * H:(kc + 1) * H],
                              in_=w2_e[kc * 128:(kc + 1) * 128, :])
        b12_s = cp.tile([1, H], BF16)
        nc.sync.dma_start(out=b12_s, in_=b12_e[:])
        vt_s = cp.tile([128, 2], BF16)
        nc.sync.dma_start(out=vt_s, in_=vt_e[:])
        xt_s = cp.tile([128, 8], I32)
        nc.sync.dma_start(out=xt_s, in_=xt_e[:])
        qi_s = cp.tile([128, 1], I32)
        nc.sync.dma_start(out=qi_s, in_=qi_e[:])
        ki_s = cp.tile([128, 4], I32)
        nc.sync.dma_start(out=ki_s, in_=ki_e[:])
        mask_s = cp.tile([128, T], F32)
        nc.sync.dma_start(out=mask_s, in_=mask_e[:])

        # ---- embedding gather + transpose -> xeT[b] [128, 2ec*512] bf16 ----
        xeT = [cp.tile([128, 2 * T], BF16, name=f"xeT{b}") for b in range(B)]
        for tch in range(4):
            for b in range(B):
                xe_rows = sp.tile([128, E], F32)
                nc.gpsimd.indirect_dma_start(
                    out=xe_rows, out_offset=None, in_=emb_e[:],
                    in_offset=bass.IndirectOffsetOnAxis(
                        ap=xt_s[:, b * 4 + tch:b * 4 + tch + 1], axis=0))
                for ec in range(2):
                    trp = pp.tile([128, T], F32, name="big", bufs=2)[:, 0:128]
                    nc.tensor.transpose(trp, xe_rows[:, ec * 128:(ec + 1) * 128], ident)
                    nc.scalar.activation(
                        xeT[b][:, ec * T + tch * 128: ec * T + (tch + 1) * 128],
                        trp, AF.Copy)

        # ---- gx precompute: gxT [128, T*16] bf16, col = t*16 + gc*2 + b ----
        # chunked over t so the LSTM can start after the first chunk
        gxT = cp.tile([128, T * 16], BF16)
        TCH = 64
        for tch in range(T // TCH):
            for b in range(B):
                for gc in range(8):
                    gx_ps = pp.tile([128, T], F32, name="big", bufs=2)[:, 0:TCH]
                    for ec in range(2):
                        nc.tensor.matmul(
                            gx_ps,
                            wih_s[:, ec * 1024 + gc * 128: ec * 1024 + (gc + 1) * 128],
                            xeT[b][:, ec * T + tch * TCH: ec * T + (tch + 1) * TCH],
                            start=(ec == 0), stop=(ec == 1))
                    off = tch * TCH * 16 + gc * 2 + b
                    nc.vector.tensor_scalar(
                        out=gxT[:, off: off + 16 * (TCH - 1) + 1: 16], in0=gx_ps,
                        scalar1=biasT_s[:, gc:gc + 1], scalar2=None,
                        op0=mybir.AluOpType.add)

        # ---- LSTM ----
        # outT_all col = t*4 + kc*2 + b (h in bf16, written directly by the
        # h-mul). act tiles (ping-pong): cols 0:4=i, 4:8=f, 8:12=o,
        # 12:16=tanh(g), 16:20=c_prev (f32). gx is injected into PSUM with an
        # identity matmul so the gate activations read PSUM directly.
        # g-gate matmuls run first so tanh(g) overlaps the remaining matmuls.
        outT_all = cp.tile([128, 4 * T], BF16)
        acts = [cp.tile([128, 20], F32, name=f"act{i}") for i in range(2)]
        nc.vector.memset(acts[0][:, 16:20], 0.0)
        z4 = cp.tile([128, 4], BF16)
        nc.vector.memset(z4, 0.0)
        # gate order in tiles (host perm): i, f, o, g -> gc blocks
        #   i: gc0,1  f: gc2,3  o: gc4,5  g: gc6,7
        # gates go to three separate PSUM tiles (banks) so each activation
        # only waits for its own gate matmuls (PSUM deps are bank-level):
        # g first (tanh overlaps i/f matmuls), then i,f, then o.
        MM_ORDER = [6, 7, 0, 1, 2, 3, 4, 5]

        def outv(b, hc, t0, n):
            """[128, n] bf16 view of outputs: h-chunk hc, batch b, t0..t0+n."""
            s = t0 * 4 + hc * 2 + b
            return outT_all[:, s: s + 4 * (n - 1) + 1: 4]

        bT_s = cp.tile([128, 2 * T], BF16)  # col = hc*512 + j
        our = [cp.tile([128, H], BF16, name=f"our{jc}") for jc in range(4)]

        def features(tch):
            """a/b features + output rows -> DRAM for t-chunk tch, then the
            key-side gathers/transposes for that chunk.

            Emitted right after the LSTM steps that produce chunk tch, so the
            scheduler can backfill everything into LSTM idle slots.
            """
            for b in range(B):
                for w_s, dram, row0, with_bias in (
                        (w1_s, a_dram, b * T + tch * 128, True),
                        (w2_s, b_dram_c[tch], b * 128, False)):
                    f_t = pp.tile([128, T], F32, name="big", bufs=2)
                    f_ps = f_t[:, 0:H]
                    for hc in range(2):
                        nc.tensor.matmul(
                            f_ps,
                            outv(b, hc, tch * 128, 128),
                            w_s[:, hc * H:(hc + 1) * H],
                            start=(hc == 0),
                            stop=(False if with_bias else hc == 1))
                    if with_bias:
                        nc.tensor.matmul(f_ps, ones_s, b12_s, start=False, stop=True)
                    f_sb = sp.tile([128, H], F32, name="f_sb", bufs=4)
                    nc.vector.tensor_copy(out=f_sb, in_=f_ps)
                    nc.sync.dma_start(out=dram[row0: row0 + 128, :], in_=f_sb)
                o_sb = sp.tile([128, H], BF16, name="o_sb", bufs=4)
                for hc in range(2):
                    trp = pp.tile([128, T], BF16, name="bigb", bufs=1)[:, 0:128]
                    nc.tensor.transpose(
                        trp, outv(b, hc, tch * 128, 128), identb)
                    nc.scalar.activation(o_sb[:, hc * 128:(hc + 1) * 128], trp, AF.Copy)
                nc.sync.dma_start(
                    out=o_dram[b * T + tch * 128: b * T + (tch + 1) * 128, :],
                    in_=o_sb)
                nc.sync.dma_start(
                    out=o_dram_c[tch][b * 128:(b + 1) * 128, :], in_=o_sb)
            # key-side gathers for this chunk (per-core batch pick via ki)
            b_rows = sp.tile([128, H], F32, name="b_rows", bufs=4)
            nc.gpsimd.indirect_dma_start(
                out=b_rows, out_offset=None, in_=b_dram_c[tch][:],
                in_offset=bass.IndirectOffsetOnAxis(ap=ki_s[:, tch:tch + 1], axis=0))
            nc.gpsimd.indirect_dma_start(
                out=our[tch], out_offset=None, in_=o_dram_c[tch][:],
                in_offset=bass.IndirectOffsetOnAxis(ap=ki_s[:, tch:tch + 1], axis=0))
            for hc in range(2):
                trp = pp.tile([128, T], F32, name="big", bufs=2)[:, 0:128]
                nc.tensor.transpose(trp, b_rows[:, hc * 128:(hc + 1) * 128], ident)
                nc.scalar.activation(
                    bT_s[:, hc * T + tch * 128: hc * T + (tch + 1) * 128],
                    trp, AF.Copy)

        for t in range(T):
            A = acts[t % 2]
            An = acts[(t + 1) % 2]
            hT = z4 if t == 0 else outT_all[:, (t - 1) * 4: t * 4]
            g_if = pp.tile([128, 8], F32, name="gps_if", bufs=1)
            g_o = pp.tile([128, 4], F32, name="gps_o", bufs=1)
            g_g = pp.tile([128, 4], F32, name="gps_g", bufs=1)

            def gview(gc):
                if gc < 4:
                    return g_if[:, gc * 2: gc * 2 + 2]
                if gc < 6:
                    return g_o[:, (gc - 4) * 2: (gc - 4) * 2 + 2]
                return g_g[:, (gc - 6) * 2: (gc - 6) * 2 + 2]

            nc.tensor.matmul(g_g, identb, gxT[:, t * 16 + 12: t * 16 + 16],
                             start=True, stop=False, skip_group_check=True)
            nc.tensor.matmul(g_if, identb, gxT[:, t * 16: t * 16 + 8],
                             start=True, stop=False, skip_group_check=True)
            nc.tensor.matmul(g_o, identb, gxT[:, t * 16 + 8: t * 16 + 12],
                             start=True, stop=False, skip_group_check=True)
            for i, gc in enumerate(MM_ORDER):
                for kc in range(2):
                    nc.tensor.matmul(
                        gview(gc),
                        whh_s[:, kc * 1024 + gc * 128: kc * 1024 + (gc + 1) * 128],
                        hT[:, kc * 2: kc * 2 + 2],
                        start=False, stop=(i == 7 and kc == 1),
                        skip_group_check=True)
            nc.scalar.activation(A[:, 12:16], g_g, AF.Tanh)
            nc.scalar.activation(A[:, 0:8], g_if, AF.Sigmoid)
            nc.scalar.activation(A[:, 8:12], g_o, AF.Sigmoid)
            prod = sp.tile([128, 8], F32, name="prod")
            nc.vector.tensor_mul(out=prod, in0=A[:, 0:8], in1=A[:, 12:20])
            nc.vector.tensor_add(out=An[:, 16:20], in0=prod[:, 0:4],
                                 in1=prod[:, 4:8])
            thc = sp.tile([128, 4], F32, name="thc")
            nc.scalar.activation(thc, An[:, 16:20], AF.Tanh)
            nc.vector.tensor_mul(out=outT_all[:, t * 4:(t + 1) * 4],
                                 in0=A[:, 8:12], in1=thc)
            if (t + 1) % 128 == 0:
                features((t + 1) // 128 - 1)

        # ---- gathers for this core's (b, iblock) ----
        aq_rows = cp.tile([128, H], F32)
        nc.gpsimd.indirect_dma_start(
            out=aq_rows, out_offset=None, in_=a_dram[:],
            in_offset=bass.IndirectOffsetOnAxis(ap=qi_s[:, 0:1], axis=0))
        oq_rows = cp.tile([128, H], BF16)
        nc.gpsimd.indirect_dma_start(
            out=oq_rows, out_offset=None, in_=o_dram[:],
            in_offset=bass.IndirectOffsetOnAxis(ap=qi_s[:, 0:1], axis=0))
        aq_s = cp.tile([128, H], BF16)    # col = hc*128 + q
        oqT_s = cp.tile([128, H], BF16)
        for hc in range(2):
            trp = pp.tile([128, T], F32, name="big", bufs=2)[:, 0:128]
            nc.tensor.transpose(trp, aq_rows[:, hc * 128:(hc + 1) * 128], ident)
            nc.scalar.activation(aq_s[:, hc * 128:(hc + 1) * 128], trp, AF.Copy)
            trp2 = pp.tile([128, T], BF16, name="bigb", bufs=1)[:, 0:128]
            nc.tensor.transpose(trp2, oq_rows[:, hc * 128:(hc + 1) * 128], identb)
            nc.scalar.activation(oqT_s[:, hc * 128:(hc + 1) * 128], trp2, AF.Copy)

        # ---- scores + softmax ----
        # q slot s holds global row 4s+ib (strided assignment), so the
        # causal key extent is slot-uniform across cores: ext(s) >= 4s+4.
        # Uncomputed score columns stay 0 and the additive mask kills them.
        sm_s = cp.tile([128, T], F32)
        nc.vector.memset(sm_s, 0.0)
        for q in range(QB):
            ext = min(T, 128 * ((4 * q + 4 + 127) // 128))
            sc1 = pp.tile([1, T], F32, name="sc1", bufs=2)[:, 0:ext]
            for hc in range(2):
                th = sp.tile([128, T], BF16, name="th", bufs=4)[:, 0:ext]
                nc.scalar.activation(
                    th, bT_s[:, hc * T: hc * T + ext], AF.Tanh,
                    bias=aq_s[:, hc * 128 + q: hc * 128 + q + 1])
                nc.tensor.matmul(sc1, vt_s[:, hc:hc + 1], th,
                                 start=(hc == 0), stop=(hc == 1))
            scq = sp.tile([1, T], F32, name="scq", bufs=4)[:, 0:ext]
            nc.vector.tensor_copy(out=scq, in_=sc1)
            nc.sync.dma_start(out=sm_s[q:q + 1, 0:ext], in_=scq)

        # ---- projection, oq half: emitted after the scores loop so it
        # backfills PE idle slots during the (ACT-bound) scores phase;
        # partial logits staged in SBUF as bf16 ----
        partial = cp.tile([128, NVB * VB], BF16)
        for vb in range(NVB):
            wt1 = wp.tile([128, 2 * VB], BF16, name="wt1")
            nc.sync.dma_start(
                out=wt1[:].rearrange("p (a v) -> p a v", a=2),
                in_=wfc_e[0:256, vb * VB:(vb + 1) * VB].rearrange(
                    "(a p) v -> p a v", p=128))
            ps = pp.tile([128, T], F32, name="big", bufs=2)[:, 0:VB]
            for kc in range(2):
                nc.tensor.matmul(ps, oqT_s[:, kc * 128:(kc + 1) * 128],
                                 wt1[:, kc * VB:(kc + 1) * VB],
                                 start=(kc == 0), stop=(kc == 1))
            nc.vector.tensor_copy(out=partial[:, vb * VB:(vb + 1) * VB], in_=ps)

        nc.vector.tensor_add(out=sm_s, in0=sm_s, in1=mask_s)
        nmx = cp.tile([128, 1], F32)
        nc.vector.reduce_max(nmx, sm_s, axis=AX.X, negate=True)
        ex_s = cp.tile([128, T], F32)
        ssum = cp.tile([128, 1], F32)
        nc.scalar.activation(ex_s, sm_s, AF.Exp, bias=nmx, accum_out=ssum)
        rs = cp.tile([128, 1], F32)
        nc.vector.reciprocal(rs, ssum)
        at_s = cp.tile([128, T], F32)
        nc.vector.tensor_scalar(out=at_s, in0=ex_s, scalar1=rs, scalar2=None,
                                op0=mybir.AluOpType.mult)

        # ---- context: ctxT [h, q] ----
        ctx_ps = pp.tile([128, T], F32, name="big", bufs=2)[:, 0:H]
        atT = [cp.tile([128, 128], BF16, name=f"atT{jc}") for jc in range(4)]
        for jc in range(4):
            trp = pp.tile([128, T], F32, name="big", bufs=2)[:, 0:128]
            nc.tensor.transpose(trp, at_s[:, jc * 128:(jc + 1) * 128], ident)
            nc.scalar.activation(atT[jc], trp, AF.Copy)
        for hc in range(2):
            for jc in range(4):
                nc.tensor.matmul(ctx_ps[:, hc * 128:(hc + 1) * 128],
                                 our[jc][:, hc * 128:(hc + 1) * 128], atT[jc],
                                 start=(jc == 0), stop=(jc == 3))
        ctxT_s = cp.tile([128, H], BF16)
        nc.vector.tensor_copy(out=ctxT_s, in_=ctx_ps)

        # ---- projection, ctx half + staged oq partial ----
        for vb in range(NVB):
            wt2 = wp.tile([128, 2 * VB], BF16, name="wt2")
            nc.scalar.dma_start(
                out=wt2[:].rearrange("p (a v) -> p a v", a=2),
                in_=wfc_e[256:512, vb * VB:(vb + 1) * VB].rearrange(
                    "(a p) v -> p a v", p=128))
            # alternate between two PSUM tags (4 banks total) for a deeper
            # matmul/add/store pipeline
            lg_ps = pp.tile([128, T], F32, name=("big" if vb % 2 else "sc1"),
                            bufs=2)[:, 0:VB]
            for kc in range(2):
                nc.tensor.matmul(lg_ps, ctxT_s[:, kc * 128:(kc + 1) * 128],
                                 wt2[:, kc * VB:(kc + 1) * VB],
                                 start=(kc == 0), stop=(kc == 1))
            lg_sb = sp.tile([128, VB], F32, name="lg_sb", bufs=4)
            nc.vector.tensor_add(out=lg_sb, in0=lg_ps,
                                 in1=partial[:, vb * VB:(vb + 1) * VB])
            nc.sync.dma_start(out=out_e[:, vb * VB:(vb + 1) * VB], in_=lg_sb)

    nc.finalize()
    return nc


_NC = None


def _get_nc():
    global _NC
    if _NC is None:
        _NC = build()
    return _NC


def _prep(inputs):
    x = np.asarray(inputs["x"])
    perm = np.concatenate([np.arange(0, 512), np.arange(768, 1024),
                           np.arange(512, 768)])
    wihT = np.ascontiguousarray(np.asarray(inputs["W_ih"])[perm].T.astype(BFNP))
    whhT = np.ascontiguousarray(np.asarray(inputs["W_hh"])[perm].T.astype(BFNP))
    bias = (np.asarray(inputs["b_ih"]) + np.asarray(inputs["b_hh"]))[perm]
    biasT = np.ascontiguousarray(bias.reshape(8, 128).T)
    w1T = np.ascontiguousarray(np.asarray(inputs["W1"]).T.astype(BFNP))
    w2T = np.ascontiguousarray(np.asarray(inputs["W2"]).T.astype(BFNP))
    b12 = (np.asarray(inputs["b1"]) + np.asarray(inputs["b2"])).reshape(1, H)
    vt = np.ascontiguousarray(np.asarray(inputs["V"])[0].reshape(2, 128).T.astype(BFNP))
    wfcT = np.ascontiguousarray(np.asarray(inputs["Wfc"]).T.astype(BFNP))
    xt = np.zeros((128, 8), np.int32)
    for b in range(B):
        for tch in range(4):
            xt[:, b * 4 + tch] = x[b, tch * 128:(tch + 1) * 128]
    common = dict(
        emb=np.ascontiguousarray(np.asarray(inputs["emb"], np.float32)),
        xt=xt, wihT=wihT, whhT=whhT,
        biasT=np.ascontiguousarray(biasT.astype(np.float32)),
        w1T=w1T, w2T=w2T,
        b12=np.ascontiguousarray(b12.astype(BFNP)), vt=vt,
        wfcT=wfcT)
    r = np.arange(128)
    in_maps = []
    for c in range(NCORES):
        b, ib = divmod(c, 4)
        qi = (b * T + 4 * r + ib).astype(np.int32).reshape(128, 1)
        ki = np.stack([(b * 128 + r).astype(np.int32)
                       for jc in range(4)], axis=1)
        mask = np.where(np.arange(T)[None, :] <= (4 * r + ib)[:, None],
                        np.float32(0.0), np.float32(-1e30)).astype(np.float32)
        m = dict(common)
        m.update(qi=qi, ki=np.ascontiguousarray(ki), mask=mask)
        in_maps.append(m)
    return in_maps


LAST = None


def assemble(results, inputs):
    bfc = np.asarray(inputs["bfc"], np.float32)
    logits = np.empty((B, T, VOCAB), np.float32)
    for c in range(NCORES):
        b, ib = divmod(c, 4)
        logits[b, ib::4, :] = results[c]["out"]
    logits += bfc[None, None, :]
    return logits


def kernel(**inputs):
    global LAST
    nc = _get_nc()
    in_maps = _prep(inputs)
    trace = bool(os.environ.get("KERNEL_TRACE"))
    try:
        br = run_bass_kernel_spmd(nc, in_maps, list(range(NCORES)), trace=trace)
    except Exception:
        if not trace:
            raise
        br = run_bass_kernel_spmd(nc, in_maps, list(range(NCORES)), trace=False)
    LAST = br
    return assemble(br.results, inputs)


if __name__ == "__main__":
    build()
    print("build ok")


# revision 35
# speedup vs baseline: 1.2775x; 1.0067x over previous
"""RNN(LSTM)+additive-attention language model on 8 trn2 cores.

Sharding: every core runs the full LSTM (both batches, merged into one set of
[128, 4] state tiles); core c = (b, ib) then does attention + vocab projection
for query rows [ib*128, (ib+1)*128) of batch b. Per-core row selection is done
with indirect-DMA gathers driven by per-core int32 index inputs, so all 8
cores run one identical SPMD program.

All matmuls run in bf16 (fp32 matmuls double-pump as HIGH/LOW instruction
pairs and disable fast-weight-load, which made LDWEIGHTS the kernel
bottleneck). Accumulation stays fp32 in PSUM; the softmax and LSTM cell
state stay fp32.
"""

import os
import numpy as np
import ml_dtypes
from contextlib import ExitStack

import concourse.bass as bass
import concourse.tile as tile
from concourse import bacc, mybir
from concourse.bass_utils import run_bass_kernel_spmd
from concourse.masks import make_identity

F32 = mybir.dt.float32
BF16 = mybir.dt.bfloat16
I32 = mybir.dt.int32
AF = mybir.ActivationFunctionType
AX = mybir.AxisListType
BFNP = ml_dtypes.bfloat16

B, T, E, H, VOCAB = 2, 512, 256, 256, 32000
NCORES = 8
QB = 128          # query rows per core
VB = 500          # vocab cols per projection block
NVB = VOCAB // VB  # 64


def build():
    nc = bacc.Bacc("TRN2", num_devices=NCORES)

    emb_e = nc.declare_dram_parameter("emb", [VOCAB, E], F32, isOutput=False)
    xt_e = nc.declare_dram_parameter("xt", [128, 8], I32, isOutput=False)
    wih_e = nc.declare_dram_parameter("wihT", [E, 4 * H], BF16, isOutput=False)
    whh_e = nc.declare_dram_parameter("whhT", [H, 4 * H], BF16, isOutput=False)
    bT_e = nc.declare_dram_parameter("biasT", [128, 8], F32, isOutput=False)
    w1_e = nc.declare_dram_parameter("w1T", [H, H], BF16, isOutput=False)
    w2_e = nc.declare_dram_parameter("w2T", [H, H], BF16, isOutput=False)
    b12_e = nc.declare_dram_parameter("b12", [1, H], BF16, isOutput=False)
    vt_e = nc.declare_dram_parameter("vt", [128, 2], BF16, isOutput=False)
    wfc_e = nc.declare_dram_parameter("wfcT", [2 * H, VOCAB], BF16, isOutput=False)
    qi_e = nc.declare_dram_parameter("qi", [128, 1], I32, isOutput=False)
    ki_e = nc.declare_dram_parameter("ki", [128, 4], I32, isOutput=False)
    mask_e = nc.declare_dram_parameter("mask", [128, T], F32, isOutput=False)
    out_e = nc.declare_dram_parameter("out", [QB, VOCAB], F32, isOutput=True)

    a_dram = nc.dram_tensor("a_scr", [B * T, H], F32)
    o_dram = nc.dram_tensor("o_scr", [B * T, H], BF16)
    # per-j-chunk scratch so key-side gathers only depend on their own
    # chunk's writes and can run during the LSTM
    b_dram_c = [nc.dram_tensor(f"b_scr{j}", [B * 128, H], F32) for j in range(4)]
    o_dram_c = [nc.dram_tensor(f"o_scr{j}", [B * 128, H], BF16) for j in range(4)]

    with tile.TileContext(nc) as tc, ExitStack() as ctx:
        cp = ctx.enter_context(tc.tile_pool(name="cp", bufs=1))
        sp = ctx.enter_context(tc.tile_pool(name="sp", bufs=3))
        wp = ctx.enter_context(tc.tile_pool(name="wp", bufs=8))
        pp = ctx.enter_context(tc.tile_pool(name="pp", bufs=2, space="PSUM"))

        # ---- constants / params ----
        ident = cp.tile([128, 128], F32)
        make_identity(nc, ident)
        identb = cp.tile([128, 128], BF16)
        nc.vector.tensor_copy(out=identb, in_=ident)
        ones_s = cp.tile([1, 128], BF16)
        nc.vector.memset(ones_s, 1.0)

        wih_s = cp.tile([128, 2 * 4 * H], BF16)   # col = kc*1024 + g
        whh_s = cp.tile([128, 2 * 4 * H], BF16)
        for kc in range(2):
            nc.sync.dma_start(out=wih_s[:, kc * 1024:(kc + 1) * 1024],
                              in_=wih_e[kc * 128:(kc + 1) * 128, :])
            nc.sync.dma_start(out=whh_s[:, kc * 1024:(kc + 1) * 1024],
                              in_=whh_e[kc * 128:(kc + 1) * 128, :])
        biasT_s = cp.tile([128, 8], F32)
        nc.sync.dma_start(out=biasT_s, in_=bT_e[:])
        w1_s = cp.tile([128, 2 * H], BF16)        # col = hcin*256 + hout
        w2_s = cp.tile([128, 2 * H], BF16)
        for kc in range(2):
            nc.sync.dma_start(out=w1_s[:, kc * H:(kc + 1) * H],
                              in_=w1_e[kc * 128:(kc + 1) * 128, :])
            nc.sync.dma_start(out=w2_s[:, kc * H:(kc + 1) * H],
                              in_=w2_e[kc * 128:(kc + 1) * 128, :])
        b12_s = cp.tile([1, H], BF16)
        nc.sync.dma_start(out=b12_s, in_=b12_e[:])
        vt_s = cp.tile([128, 2], BF16)
        nc.sync.dma_start(out=vt_s, in_=vt_e[:])
        xt_s = cp.tile([128, 8], I32)
        nc.sync.dma_start(out=xt_s, in_=xt_e[:])
        qi_s = cp.tile([128, 1], I32)
        nc.sync.dma_start(out=qi_s, in_=qi_e[:])
        ki_s = cp.tile([128, 4], I32)
        nc.sync.dma_start(out=ki_s, in_=ki_e[:])
        mask_s = cp.tile([128, T], F32)
        nc.sync.dma_start(out=mask_s, in_=mask_e[:])

        # ---- embedding gather + transpose -> xeT[b] [128, 2ec*512] bf16 ----
        xeT = [cp.tile([128, 2 * T], BF16, name=f"xeT{b}") for b in range(B)]
        for tch in range(4):
            for b in range(B):
                xe_rows = sp.tile([128, E], F32)
                nc.gpsimd.indirect_dma_start(
                    out=xe_rows, out_offset=None, in_=emb_e[:],
                    in_offset=bass.IndirectOffsetOnAxis(
                        ap=xt_s[:, b * 4 + tch:b * 4 + tch + 1], axis=0))
                for ec in range(2):
                    trp = pp.tile([128, T], F32, name="big", bufs=2)[:, 0:128]
                    nc.tensor.transpose(trp, xe_rows[:, ec * 128:(ec + 1) * 128], ident)
                    nc.scalar.activation(
                        xeT[b][:, ec * T + tch * 128: ec * T + (tch + 1) * 128],
                        trp, AF.Copy)

        # ---- gx precompute: gxT [128, T*16] bf16, col = t*16 + gc*2 + b ----
        # chunked over t so the LSTM can start after the first chunk
        gxT = cp.tile([128, T * 16], BF16)
        TCH = 64
        for tch in range(T // TCH):
            for b in range(B):
                for gc in range(8):
                    gx_ps = pp.tile([128, T], F32, name="big", bufs=2)[:, 0:TCH]
                    for ec in range(2):
                        nc.tensor.matmul(
                            gx_ps,
                            wih_s[:, ec * 1024 + gc * 128: ec * 1024 + (gc + 1) * 128],
                            xeT[b][:, ec * T + tch * TCH: ec * T + (tch + 1) * TCH],
                            start=(ec == 0), stop=(ec == 1))
                    off = tch * TCH * 16 + gc * 2 + b
                    nc.vector.tensor_scalar(
                        out=gxT[:, off: off + 16 * (TCH - 1) + 1: 16], in0=gx_ps,
                        scalar1=biasT_s[:, gc:gc + 1], scalar2=None,
                        op0=mybir.AluOpType.add)

        # ---- LSTM ----
        # outT_all col = t*4 + kc*2 + b (h in bf16, written directly by the
        # h-mul). act tiles (ping-pong): cols 0:4=i, 4:8=f, 8:12=o,
        # 12:16=tanh(g), 16:20=c_prev (f32). gx is injected into PSUM with an
        # identity matmul so the gate activations read PSUM directly.
        # g-gate matmuls run first so tanh(g) overlaps the remaining matmuls.
        outT_all = cp.tile([128, 4 * T], BF16)
        acts = [cp.tile([128, 20], F32, name=f"act{i}") for i in range(2)]
        nc.vector.memset(acts[0][:, 16:20], 0.0)
        z4 = cp.tile([128, 4], BF16)
        nc.vector.memset(z4, 0.0)
        # gate order in tiles (host perm): i, f, o, g -> gc blocks
        #   i: gc0,1  f: gc2,3  o: gc4,5  g: gc6,7
        # gates go to three separate PSUM tiles (banks) so each activation
        # only waits for its own gate matmuls (PSUM deps are bank-level):
        # g first (tanh overlaps i/f matmuls), then i,f, then o.
        MM_ORDER = [6, 7, 0, 1, 2, 3, 4, 5]

        def outv(b, hc, t0, n):
            """[128, n] bf16 view of outputs: h-chunk hc, batch b, t0..t0+n."""
            s = t0 * 4 + hc * 2 + b
            return outT_all[:, s: s + 4 * (n - 1) + 1: 4]

        bT_s = cp.tile([128, 2 * T], BF16)  # col = hc*512 + j
        our = [cp.tile([128, H], BF16, name=f"our{jc}") for jc in range(4)]

        def features(tch):
            """a/b features + output rows -> DRAM for t-chunk tch, then the
            key-side gathers/transposes for that chunk.

            Emitted right after the LSTM steps that produce chunk tch, so the
            scheduler can backfill everything into LSTM idle slots.
            """
            for b in range(B):
                for w_s, dram, row0, with_bias in (
                        (w1_s, a_dram, b * T + tch * 128, True),
                        (w2_s, b_dram_c[tch], b * 128, False)):
                    f_t = pp.tile([128, T], F32, name="big", bufs=2)
                    f_ps = f_t[:, 0:H]
                    for hc in range(2):
                        nc.tensor.matmul(
                            f_ps,
                            outv(b, hc, tch * 128, 128),
                            w_s[:, hc * H:(hc + 1) * H],
                            start=(hc == 0),
                            stop=(False if with_bias else hc == 1))
                    if with_bias:
                        nc.tensor.matmul(f_ps, ones_s, b12_s, start=False, stop=True)
                    f_sb = sp.tile([128, H], F32, name="f_sb", bufs=4)
                    nc.vector.tensor_copy(out=f_sb, in_=f_ps)
                    nc.sync.dma_start(out=dram[row0: row0 + 128, :], in_=f_sb)
                o_sb = sp.tile([128, H], BF16, name="o_sb", bufs=4)
                for hc in range(2):
                    trp = pp.tile([128, T], BF16, name="bigb", bufs=1)[:, 0:128]
                    nc.tensor.transpose(
                        trp, outv(b, hc, tch * 128, 128), identb)
                    nc.scalar.activation(o_sb[:, hc * 128:(hc + 1) * 128], trp, AF.Copy)
                nc.sync.dma_start(
                    out=o_dram[b * T + tch * 128: b * T + (tch + 1) * 128, :],
                    in_=o_sb)
                nc.sync.dma_start(
                    out=o_dram_c[tch][b * 128:(b + 1) * 128, :], in_=o_sb)
            # key-side gathers for this chunk (per-core batch pick via ki)
            b_rows = sp.tile([128, H], F32, name="b_rows", bufs=4)
            nc.gpsimd.indirect_dma_start(
                out=b_rows, out_offset=None, in_=b_dram_c[tch][:],
                in_offset=bass.IndirectOffsetOnAxis(ap=ki_s[:, tch:tch + 1], axis=0))
            nc.gpsimd.indirect_dma_start(
                out=our[tch], out_offset=None, in_=o_dram_c[tch][:],
                in_offset=bass.IndirectOffsetOnAxis(ap=ki_s[:, tch:tch + 1], axis=0))
            for hc in range(2):
                trp = pp.tile([128, T], F32, name="big", bufs=2)[:, 0:128]
                nc.tensor.transpose(trp, b_rows[:, hc * 128:(hc + 1) * 128], ident)
                nc.scalar.activation(
                    bT_s[:, hc * T + tch * 128: hc * T + (tch + 1) * 128],
                    trp, AF.Copy)

        for t in range(T):
            A = acts[t % 2]
            An = acts[(t + 1) % 2]
            hT = z4 if t == 0 else outT_all[:, (t - 1) * 4: t * 4]
            g_if = pp.tile([128, 8], F32, name="gps_if", bufs=1)
            g_o = pp.tile([128, 4], F32, name="gps_o", bufs=1)
            g_g = pp.tile([128, 4], F32, name="gps_g", bufs=1)

            def gview(gc):
                if gc < 4:
                    return g_if[:, gc * 2: gc * 2 + 2]
                if gc < 6:
                    return g_o[:, (gc - 4) * 2: (gc - 4) * 2 + 2]
                return g_g[:, (gc - 6) * 2: (gc - 6) * 2 + 2]

            nc.tensor.matmul(g_g, identb, gxT[:, t * 16 + 12: t * 16 + 16],
                             start=True, stop=False, skip_group_check=True)
            nc.tensor.matmul(g_if, identb, gxT[:, t * 16: t * 16 + 8],
                             start=True, stop=False, skip_group_check=True)
            nc.tensor.matmul(g_o, identb, gxT[:, t * 16 + 8: t * 16 + 12],
                             start=True, stop=False, skip_group_check=True)
            for i, gc in enumerate(MM_ORDER):
                for kc in range(2):
                    nc.tensor.matmul(
                        gview(gc),
                        whh_s[:, kc * 1024 + gc * 128: kc * 1024 + (gc + 1) * 128],
                        hT[:, kc * 2: kc * 2 + 2],
                        start=False, stop=(i == 7 and kc == 1),
                        skip_group_check=True)
            nc.scalar.activation(A[:, 12:16], g_g, AF.Tanh)
            nc.scalar.activation(A[:, 0:8], g_if, AF.Sigmoid)
            nc.scalar.activation(A[:, 8:12], g_o, AF.Sigmoid)
            prod = sp.tile([128, 8], F32, name="prod")
            nc.vector.tensor_mul(out=prod, in0=A[:, 0:8], in1=A[:, 12:20])
            nc.vector.tensor_add(out=An[:, 16:20], in0=prod[:, 0:4],
                                 in1=prod[:, 4:8])
            thc = sp.tile([128, 4], F32, name="thc")
            nc.scalar.activation(thc, An[:, 16:20], AF.Tanh)
            nc.vector.tensor_mul(out=outT_all[:, t * 4:(t + 1) * 4],
                                 in0=A[:, 8:12], in1=thc)
            if (t + 1) % 128 == 0:
                features((t + 1) // 128 - 1)

        # ---- gathers for this core's (b, iblock) ----
        aq_rows = cp.tile([128, H], F32)
        nc.gpsimd.indirect_dma_start(
            out=aq_rows, out_offset=None, in_=a_dram[:],
            in_offset=bass.IndirectOffsetOnAxis(ap=qi_s[:, 0:1], axis=0))
        oq_rows = cp.tile([128, H], BF16)
        nc.gpsimd.indirect_dma_start(
            out=oq_rows, out_offset=None, in_=o_dram[:],
            in_offset=bass.IndirectOffsetOnAxis(ap=qi_s[:, 0:1], axis=0))
        aq_s = cp.tile([128, H], BF16)    # col = hc*128 + q
        oqT_s = cp.tile([128, H], BF16)
        for hc in range(2):
            trp = pp.tile([128, T], F32, name="big", bufs=2)[:, 0:128]
            nc.tensor.transpose(trp, aq_rows[:, hc * 128:(hc + 1) * 128], ident)
            nc.scalar.activation(aq_s[:, hc * 128:(hc + 1) * 128], trp, AF.Copy)
            trp2 = pp.tile([128, T], BF16, name="bigb", bufs=1)[:, 0:128]
            nc.tensor.transpose(trp2, oq_rows[:, hc * 128:(hc + 1) * 128], identb)
            nc.scalar.activation(oqT_s[:, hc * 128:(hc + 1) * 128], trp2, AF.Copy)

        # ---- scores + softmax ----
        # q slot s holds global row 4s+ib (strided assignment), so the
        # causal key extent is slot-uniform across cores: ext(s) >= 4s+4.
        # Uncomputed score columns stay 0 and the additive mask kills them.
        sm_s = cp.tile([128, T], F32)
        nc.vector.memset(sm_s, 0.0)
        for q in range(QB):
            ext = min(T, 128 * ((4 * q + 4 + 127) // 128))
            sc1 = pp.tile([1, T], F32, name="sc1", bufs=2)[:, 0:ext]
            for hc in range(2):
                th = sp.tile([128, T], BF16, name="th", bufs=4)[:, 0:ext]
                nc.scalar.activation(
                    th, bT_s[:, hc * T: hc * T + ext], AF.Tanh,
                    bias=aq_s[:, hc * 128 + q: hc * 128 + q + 1])
                nc.tensor.matmul(sc1, vt_s[:, hc:hc + 1], th,
                                 start=(hc == 0), stop=(hc == 1))
            scq = sp.tile([1, T], F32, name="scq", bufs=4)[:, 0:ext]
            nc.vector.tensor_copy(out=scq, in_=sc1)
            # SWDGE queue: keeps the Sync sequencer free for weight/out DMAs
            nc.gpsimd.dma_start(out=sm_s[q:q + 1, 0:ext], in_=scq)

        # ---- projection, oq half: emitted after the scores loop so it
        # backfills PE idle slots during the (ACT-bound) scores phase;
        # partial logits staged in SBUF as bf16 ----
        partial = cp.tile([128, NVB * VB], BF16)
        for vb in range(NVB):
            wt1 = wp.tile([128, 2 * VB], BF16, name="wt1")
            nc.sync.dma_start(
                out=wt1[:].rearrange("p (a v) -> p a v", a=2),
                in_=wfc_e[0:256, vb * VB:(vb + 1) * VB].rearrange(
                    "(a p) v -> p a v", p=128))
            ps = pp.tile([128, T], F32, name="big", bufs=2)[:, 0:VB]
            for kc in range(2):
                nc.tensor.matmul(ps, oqT_s[:, kc * 128:(kc + 1) * 128],
                                 wt1[:, kc * VB:(kc + 1) * VB],
                                 start=(kc == 0), stop=(kc == 1))
            nc.vector.tensor_copy(out=partial[:, vb * VB:(vb + 1) * VB], in_=ps)

        nc.vector.tensor_add(out=sm_s, in0=sm_s, in1=mask_s)
        nmx = cp.tile([128, 1], F32)
        nc.vector.reduce_max(nmx, sm_s, axis=AX.X, negate=True)
        ex_s = cp.tile([128, T], F32)
        ssum = cp.tile([128, 1], F32)
        nc.scalar.activation(ex_s, sm_s, AF.Exp, bias=nmx, accum_out=ssum)
        rs = cp.tile([128, 1], F32)
        nc.vector.reciprocal(rs, ssum)
        at_s = cp.tile([128, T], F32)
        nc.vector.tensor_scalar(out=at_s, in0=ex_s, scalar1=rs, scalar2=None,
                                op0=mybir.AluOpType.mult)

        # ---- context: ctxT [h, q] ----
        ctx_ps = pp.tile([128, T], F32, name="big", bufs=2)[:, 0:H]
        atT = [cp.tile([128, 128], BF16, name=f"atT{jc}") for jc in range(4)]
        for jc in range(4):
            trp = pp.tile([128, T], F32, name="big", bufs=2)[:, 0:128]
            nc.tensor.transpose(trp, at_s[:, jc * 128:(jc + 1) * 128], ident)
            nc.scalar.activation(atT[jc], trp, AF.Copy)
        for hc in range(2):
            for jc in range(4):
                nc.tensor.matmul(ctx_ps[:, hc * 128:(hc + 1) * 128],
                                 our[jc][:, hc * 128:(hc + 1) * 128], atT[jc],
                                 start=(jc == 0), stop=(jc == 3))
        ctxT_s = cp.tile([128, H], BF16)
        nc.vector.tensor_copy(out=ctxT_s, in_=ctx_ps)

        # ---- projection, ctx half + staged oq partial ----
        for vb in range(NVB):
            wt2 = wp.tile([128, 2 * VB], BF16, name="wt2")
            nc.gpsimd.dma_start(
                out=wt2[:].rearrange("p (a v) -> p a v", a=2),
                in_=wfc_e[256:512, vb * VB:(vb + 1) * VB].rearrange(
                    "(a p) v -> p a v", p=128))
            # alternate between two PSUM tags (4 banks total) for a deeper
            # matmul/add/store pipeline
            lg_ps = pp.tile([128, T], F32, name=("big" if vb % 2 else "sc1"),
                            bufs=2)[:, 0:VB]
            for kc in range(2):
                nc.tensor.matmul(lg_ps, ctxT_s[:, kc * 128:(kc + 1) * 128],
                                 wt2[:, kc * VB:(kc + 1) * VB],
                                 start=(kc == 0), stop=(kc == 1))
            lg_sb = sp.tile([128, VB], F32, name="lg_sb", bufs=4)
            nc.vector.tensor_add(out=lg_sb, in0=lg_ps,
                                 in1=partial[:, vb * VB:(vb + 1) * VB])
            nc.sync.dma_start(out=out_e[:, vb * VB:(vb + 1) * VB], in_=lg_sb)

    nc.finalize()
    return nc


_NC = None


def _get_nc():
    global _NC
    if _NC is None:
        _NC = build()
    return _NC


def _prep(inputs):
    x = np.asarray(inputs["x"])
    perm = np.concatenate([np.arange(0, 512), np.arange(768, 1024),
                           np.arange(512, 768)])
    wihT = np.ascontiguousarray(np.asarray(inputs["W_ih"])[perm].T.astype(BFNP))
    whhT = np.ascontiguousarray(np.asarray(inputs["W_hh"])[perm].T.astype(BFNP))
    bias = (np.asarray(inputs["b_ih"]) + np.asarray(inputs["b_hh"]))[perm]
    biasT = np.ascontiguousarray(bias.reshape(8, 128).T)
    w1T = np.ascontiguousarray(np.asarray(inputs["W1"]).T.astype(BFNP))
    w2T = np.ascontiguousarray(np.asarray(inputs["W2"]).T.astype(BFNP))
    b12 = (np.asarray(inputs["b1"]) + np.asarray(inputs["b2"])).reshape(1, H)
    vt = np.ascontiguousarray(np.asarray(inputs["V"])[0].reshape(2, 128).T.astype(BFNP))
    wfcT = np.ascontiguousarray(np.asarray(inputs["Wfc"]).T.astype(BFNP))
    xt = np.zeros((128, 8), np.int32)
    for b in range(B):
        for tch in range(4):
            xt[:, b * 4 + tch] = x[b, tch * 128:(tch + 1) * 128]
    common = dict(
        emb=np.ascontiguousarray(np.asarray(inputs["emb"], np.float32)),
        xt=xt, wihT=wihT, whhT=whhT,
        biasT=np.ascontiguousarray(biasT.astype(np.float32)),
        w1T=w1T, w2T=w2T,
        b12=np.ascontiguousarray(b12.astype(BFNP)), vt=vt,
        wfcT=wfcT)
    r = np.arange(128)
    in_maps = []
    for c in range(NCORES):
        b, ib = divmod(c, 4)
        qi = (b * T + 4 * r + ib).astype(np.int32).reshape(128, 1)
        ki = np.stack([(b * 128 + r).astype(np.int32)
                       for jc in range(4)], axis=1)
        mask = np.where(np.arange(T)[None, :] <= (4 * r + ib)[:, None],
                        np.float32(0.0), np.float32(-1e30)).astype(np.float32)
        m = dict(common)
        m.update(qi=qi, ki=np.ascontiguousarray(ki), mask=mask)
        in_maps.append(m)
    return in_maps


LAST = None


def assemble(results, inputs):
    bfc = np.asarray(inputs["bfc"], np.float32)
    logits = np.empty((B, T, VOCAB), np.float32)
    for c in range(NCORES):
        b, ib = divmod(c, 4)
        logits[b, ib::4, :] = results[c]["out"]
    logits += bfc[None, None, :]
    return logits


def kernel(**inputs):
    global LAST
    nc = _get_nc()
    in_maps = _prep(inputs)
    trace = bool(os.environ.get("KERNEL_TRACE"))
    try:
        br = run_bass_kernel_spmd(nc, in_maps, list(range(NCORES)), trace=trace)
    except Exception:
        if not trace:
            raise
        br = run_bass_kernel_spmd(nc, in_maps, list(range(NCORES)), trace=False)
    LAST = br
    return assemble(br.results, inputs)


if __name__ == "__main__":
    build()
    print("build ok")
